# revision 1
# baseline (speedup 1.0000x reference)
"""CopyMechanism (pointer-generator) kernel for 8 Trainium2 NeuronCores.

Full problem: B=16, T=128, H=512, V=32000, S=400.
  gen = sigmoid(ctx@wh + hid@ws + trg@wx + b)          [B,T,1]
  out = gen * vocab_dists; out[b,t,ids[b,t,s]] += (1-gen)*attn[b,t,s]

Sharding: data-parallel over batch. Core i handles batches [2i, 2i+1]
(256 rows of T-steps). Weights replicated. No cross-core communication.

Device algorithm (per core, per row r):
  Decompose vocab index v = p*250 + f  (V = 128*250), so a row's 32000-wide
  output is an SBUF tile [128 partitions, 250 free].  The scatter-add of the
  S=400 attn values becomes a sum of outer products:
     M[p,f] = sum_s onehot(pi[s])[p] * (onehot(fi[s])[f] * val[s])
  computed by TensorE matmuls contracting s (4 chunks of <=128 on the
  partition axis).  One-hots are built on VectorE with iota/is_equal
  tensor_scalar ops in bf16 (indices pre-decomposed and pre-transposed on
  host -- integer-only preprocessing); A carries the scatter value.  The
  base p_gen*vocab is a 5th fp32 matmul with lhsT = p_gen*I (diagonal,
  built per row on ScalarE) that starts the PSUM accumulation group;
  ScalarE copies PSUM->SBUF and the store goes out on the ACT HWDGE ring
  (loads on the SP ring) so loads and stores don't serialize on one FIFO.

  p_gen is computed on-device (dot products + sigmoid), bounced through a
  DRAM scratch and re-loaded with a partition-broadcast AP so each row's
  scalar is available on all 128 partitions.
"""

import numpy as np
from ml_dtypes import bfloat16

# ---------------------------------------------------------------------------
# problem constants (hardcoded per contract)
B, T, H, V, S = 16, 128, 512, 32000, 400
N_CORES = 8
BPC = B // N_CORES          # batches per core
R_FULL = BPC * T            # rows per core = 256
FD_FULL = V // 128          # 250
SP_FULL = (S + 127) // 128  # 4 s-chunks
G_FULL = 16                 # rows per vocab DMA group

_PROGRAM_CACHE = {}


def build_program(R=R_FULL, FD=FD_FULL, SP=SP_FULL, G=G_FULL, mode="diag",
                  rep=1, a_engine="dve", ablate="full", pair_psum=True):
    """Build + compile the per-core Bass program. Same program for all cores.

    R : rows per core (multiple of 128)
    FD: free-dim width of the vocab decomposition (V_local = 128*FD)
    SP: number of 128-wide s-chunks (S padded to SP*128)
    G : rows per vocab/out DMA group
    mode: "diag" -> base p_gen*vocab via a diagonal matmul starting the PSUM
          group; "dve" -> base+merge on VectorE after the scatter matmuls.
    rep : repeat the whole body rep times (identical output; used for
          differential device-time measurement).
    """
    key = (R, FD, SP, G, mode, rep, a_engine, ablate, pair_psum)
    if key in _PROGRAM_CACHE:
        return _PROGRAM_CACHE[key]

    from contextlib import ExitStack

    import concourse.bass as bass
    import concourse.tile as tile
    from concourse import bacc, mybir

    f32 = mybir.dt.float32
    bf16 = mybir.dt.bfloat16
    Alu = mybir.AluOpType
    Act = mybir.ActivationFunctionType
    VL = 128 * FD
    RB = R // 128
    NG = R // G
    assert R % 128 == 0 and R % G == 0

    nc = bacc.Bacc("TRN2", target_bir_lowering=False, debug=False)

    ctx_d = nc.dram_tensor("ctx", [R, H], f32, kind="ExternalInput")
    hid_d = nc.dram_tensor("hid", [R, H], f32, kind="ExternalInput")
    trg_d = nc.dram_tensor("trg", [R, H], f32, kind="ExternalInput")
    vocab_d = nc.dram_tensor("vocab", [R, VL], f32, kind="ExternalInput")
    attnT_d = nc.dram_tensor("attnT", [128, RB * SP, 128], f32, kind="ExternalInput")
    piT_d = nc.dram_tensor("piT", [128, RB * SP, 128], f32, kind="ExternalInput")
    fiT_d = nc.dram_tensor("fiT", [128, RB * SP, 128], f32, kind="ExternalInput")
    # weights replicated across partitions on host (pure data movement)
    wh_d = nc.dram_tensor("wh", [128, H], f32, kind="ExternalInput")
    ws_d = nc.dram_tensor("ws", [128, H], f32, kind="ExternalInput")
    wx_d = nc.dram_tensor("wx", [128, H], f32, kind="ExternalInput")
    wxb_d = nc.dram_tensor("wxb", [128, 1], f32, kind="ExternalInput")
    iotaP_d = nc.dram_tensor("iotaP", [128, 128], bf16, kind="ExternalInput")
    iotaF_d = nc.dram_tensor("iotaF", [128, FD], bf16, kind="ExternalInput")
    ident_d = nc.dram_tensor("ident", [128, 128], f32, kind="ExternalInput")
    out_d = nc.dram_tensor("out", [R, VL], f32, kind="ExternalOutput")

    with tile.TileContext(nc) as tc, ExitStack() as es:
        singles = es.enter_context(tc.tile_pool(name="singles", bufs=1))
        ph1 = es.enter_context(tc.tile_pool(name="ph1", bufs=2))
        gbufs = 3 if G <= 16 else 2
        vpool = es.enter_context(tc.tile_pool(name="vpool", bufs=gbufs))
        opool = es.enter_context(tc.tile_pool(name="opool", bufs=gbufs))
        abpool = es.enter_context(tc.tile_pool(name="ab", bufs=6))
        ppool = es.enter_context(tc.tile_pool(name="psum", bufs=8, space="PSUM"))
        dpool = es.enter_context(tc.tile_pool(name="dram", bufs=1, space="DRAM"))

        # --- constants / small inputs ---
        attnT = singles.tile([128, RB * SP, 128], f32)
        nc.sync.dma_start(attnT[:], attnT_d[:])
        piT = singles.tile([128, RB * SP, 128], f32)
        nc.sync.dma_start(piT[:], piT_d[:])
        fiT = singles.tile([128, RB * SP, 128], f32)
        nc.sync.dma_start(fiT[:], fiT_d[:])
        iotaP = singles.tile([128, 128], bf16)
        nc.sync.dma_start(iotaP[:], iotaP_d[:])
        iotaF = singles.tile([128, FD], bf16)
        nc.sync.dma_start(iotaF[:], iotaF_d[:])
        ident = singles.tile([128, 128], f32)
        nc.sync.dma_start(ident[:], ident_d[:])
        wh = singles.tile([128, H], f32)
        nc.sync.dma_start(wh[:], wh_d[:])
        ws = singles.tile([128, H], f32)
        nc.sync.dma_start(ws[:], ws_d[:])
        wx = singles.tile([128, H], f32)
        nc.sync.dma_start(wx[:], wx_d[:])
        wxb = singles.tile([128, 1], f32)
        nc.sync.dma_start(wxb[:], wxb_d[:])
        scaledT = singles.tile([128, RB * SP, 128], f32)
        pgen_all = singles.tile([128, R], f32)
        om_all = singles.tile([128, R], f32)
        pgen_dram = dpool.tile([R, 1], f32)

        # --- phase 1a: p_gen per row (rows on partitions), bounce to DRAM ---
        def _phase1a():
          for blk in range(RB):
            rows = slice(blk * 128, (blk + 1) * 128)
            gacc = ph1.tile([128, 1], f32, tag="gacc")
            gtmp = ph1.tile([128, 1], f32, tag="gtmp")
            g2 = ph1.tile([128, 1], f32, tag="g2")
            prod = ph1.tile([128, H], f32, tag="prod")
            for i, (src_d, w) in enumerate(
                ((ctx_d, wh), (hid_d, ws), (trg_d, wx))
            ):
                x = ph1.tile([128, H], f32, tag="x")
                nc.sync.dma_start(x[:], src_d[rows, :])
                nc.vector.tensor_tensor(prod[:], x[:], w[:], op=Alu.mult)
                dst = (gacc, gtmp, g2)[i]
                nc.vector.tensor_reduce(
                    dst[:], prod[:], axis=mybir.AxisListType.X, op=Alu.add
                )
            gsum = ph1.tile([128, 1], f32, tag="gsum")
            nc.vector.tensor_tensor(gsum[:], gacc[:], gtmp[:], op=Alu.add)
            gall = ph1.tile([128, 1], f32, tag="gall")
            nc.vector.tensor_tensor(gall[:], gsum[:], g2[:], op=Alu.add)
            pgen_col = ph1.tile([128, 1], f32, tag="pgen")
            nc.scalar.activation(
                pgen_col[:], gall[:], Act.Sigmoid, bias=wxb[:], scale=1.0
            )
            nc.sync.dma_start(pgen_dram[rows, :], pgen_col[:])

        # --- phase 1b: broadcast p_gen to all partitions; scaled attnT ---
        def _phase1b():
            pg_flat = pgen_dram[:, 0]
            pg_bcast = bass.AP(
                tensor=pg_flat.tensor, offset=pg_flat.offset,
                ap=[[0, 128]] + list(pg_flat.ap),
            )
            nc.gpsimd.dma_start(pgen_all[:], pg_bcast)
            nc.vector.tensor_scalar(
                om_all[:], pgen_all[:], -1.0, 1.0, Alu.mult, Alu.add
            )
            for blk in range(RB):
                for c in range(SP):
                    nc.vector.tensor_tensor(
                        scaledT[:, blk * SP + c, :],
                        attnT[:, blk * SP + c, :],
                        om_all[:, blk * 128:(blk + 1) * 128],
                        op=Alu.mult,
                    )

        # --- phase 2: per-row scatter-add via one-hot matmuls ---
        vocab_v = vocab_d[:].rearrange("r (p f) -> p r f", p=128)
        out_v = out_d[:].rearrange("r (p f) -> p r f", p=128)

        def _phase2():
          for grp in range(NG):
            gr = slice(grp * G, (grp + 1) * G)
            ot = opool.tile([128, G, FD], f32)
            if mode == "dma":
                # Pre-fill ot with per-row p_gen, then the vocab load DMA
                # multiplies in transit: ot = p_gen * vocab (no PE/DVE time).
                for j in range(G):
                    r = grp * G + j
                    # ot[:, j, :] = 0*iotaF + p_gen[r]  (no broadcast APs)
                    nc.scalar.activation(
                        ot[:, j, :], iotaF[:], Act.Identity,
                        bias=pgen_all[:, r:r + 1], scale=0.0,
                    )
                nc.gpsimd.dma_start(
                    ot[:], vocab_v[:, gr, :], accum_op=Alu.mult
                )
            else:
                vt = vpool.tile([128, G, FD], f32)
                nc.sync.dma_start(vt[:], vocab_v[:, gr, :])
            if ablate == "dmaonly":
                if mode != "dma":
                    nc.scalar.copy(ot[:, :, :], vt[:, :, :])
                nc.scalar.dma_start(out_v[:, gr, :], ot[:])
                continue
            psb = None
            for j in range(G):
                r = grp * G + j
                blk = r // 128
                rl = r % 128
                if ablate == "nomm":
                    nc.scalar.copy(ot[:, j, :], vt[:, j, :])
                if pair_psum and mode == "diag":
                    if j % 2 == 0:
                        psb = ppool.tile([128, 2, 256], f32)
                    ps = psb[:, j % 2, 0:FD]
                else:
                    ps = ppool.tile([128, FD], f32)[:]
                pg_sc = pgen_all[:, r:r + 1]
                if ablate == "nomm":
                    for c in range(SP):
                        ch = blk * SP + c
                        A = abpool.tile([128, 128], bf16, tag="A")
                        eng = nc.gpsimd if a_engine == "gpsimd" else nc.vector
                        eng.tensor_scalar(
                            A[:], iotaP[:], piT[:, ch, rl:rl + 1],
                            scaledT[:, ch, rl:rl + 1], Alu.is_equal, Alu.mult,
                        )
                        Bt = abpool.tile([128, FD], bf16, tag="B")
                        nc.vector.tensor_scalar(
                            Bt[:], iotaF[:], fiT[:, ch, rl:rl + 1], None,
                            Alu.is_equal,
                        )
                    continue
                if mode == "diag":
                    D = abpool.tile([128, 128], f32, tag="D")
                    nc.scalar.mul(D[:], ident[:], pg_sc)
                    nc.tensor.matmul(
                        ps, lhsT=D[:], rhs=vt[:, j, :],
                        start=(j % 2 == 0 or not pair_psum), stop=False,
                    )
                for c in range(SP):
                    ch = blk * SP + c
                    # A carries the value: A[s,p] = (pi[s]==p) * val[s]
                    A = abpool.tile([128, 128], bf16, tag="A")
                    a_eng = nc.gpsimd if a_engine == "gpsimd" else nc.vector
                    a_eng.tensor_scalar(
                        A[:], iotaP[:], piT[:, ch, rl:rl + 1],
                        scaledT[:, ch, rl:rl + 1], Alu.is_equal, Alu.mult,
                    )
                    # B is the pure one-hot of fi (1-op, wide)
                    Bt = abpool.tile([128, FD], bf16, tag="B")
                    nc.vector.tensor_scalar(
                        Bt[:], iotaF[:], fiT[:, ch, rl:rl + 1], None,
                        Alu.is_equal,
                    )
                    last = (c == SP - 1) and (
                        not (pair_psum and mode == "diag") or j % 2 == 1
                    )
                    nc.tensor.matmul(
                        ps, lhsT=A[:], rhs=Bt[:],
                        start=(False if mode == "diag" else c == 0),
                        stop=last,
                    )
                if mode == "diag":
                    if pair_psum:
                        if j % 2 == 1:
                            nc.scalar.copy(
                                ot[:, j - 1:j + 1, :], psb[:, :, 0:FD]
                            )
                    else:
                        nc.scalar.copy(ot[:, j, :], ps)
                elif mode == "dma":
                    nc.vector.tensor_tensor(
                        ot[:, j, :], ot[:, j, :], ps[:], op=Alu.add
                    )
                else:
                    nc.vector.tensor_scalar(
                        ot[:, j, :], vt[:, j, :], pg_sc, None, Alu.mult
                    )
                    nc.vector.tensor_tensor(
                        ot[:, j, :], ot[:, j, :], ps[:], op=Alu.add
                    )
            nc.scalar.dma_start(out_v[:, gr, :], ot[:])

        for _ in range(rep):
            _phase1a()
            _phase1b()
            _phase2()

    nc.compile()
    _PROGRAM_CACHE[key] = nc
    return nc


def make_core_inputs(ctx, hid, trg, vocab, attn, ids, w_h, w_s, w_x_w, w_x_b,
                     R=R_FULL, FD=FD_FULL, SP=SP_FULL):
    """Host-side prep for one core: flatten rows, decompose + transpose indices.

    ctx/hid/trg: [R, H] f32; vocab: [R, 128*FD] f32; attn: [R, S'] f32;
    ids: [R, S'] int. Returns the in_map dict for this core.
    """
    RB = R // 128
    Sp = SP * 128
    Sl = attn.shape[1]
    f32 = np.float32

    ids = np.asarray(ids).astype(np.int64)
    pi = (ids // FD).astype(f32)
    fi = (ids % FD).astype(f32)

    def tr(x, pad):
        full = np.full((R, Sp), pad, dtype=f32)
        full[:, :Sl] = x
        # [R, Sp] -> [RB, 128(r), SP, 128(s)] -> [s, RB, SP, r]
        t = full.reshape(RB, 128, SP, 128).transpose(3, 0, 2, 1)
        return np.ascontiguousarray(t.reshape(128, RB * SP, 128))

    def rep(w, n):
        return np.ascontiguousarray(
            np.broadcast_to(np.asarray(w, dtype=f32).reshape(1, n), (128, n))
        )

    return {
        "ctx": np.ascontiguousarray(ctx, dtype=f32),
        "hid": np.ascontiguousarray(hid, dtype=f32),
        "trg": np.ascontiguousarray(trg, dtype=f32),
        "vocab": np.ascontiguousarray(vocab, dtype=f32),
        "attnT": tr(np.asarray(attn, dtype=f32), 0.0),
        "piT": tr(pi, 1.0e4),
        "fiT": tr(fi, -1.0),
        "wh": rep(w_h, H),
        "ws": rep(w_s, H),
        "wx": rep(w_x_w, H),
        "wxb": rep(w_x_b, 1),
        "iotaP": rep(np.arange(128, dtype=f32), 128).astype(bfloat16),
        "iotaF": rep(np.arange(FD, dtype=f32), FD).astype(bfloat16),
        "ident": np.eye(128, dtype=f32),
    }


def make_in_maps(context_vecs, hidden, trg_embs, vocab_dists, attn_dists,
                 src_ids, w_h, w_s, w_x_w, w_x_b):
    """Build the 8 per-core input dicts from full inputs."""
    context_vecs = np.asarray(context_vecs)
    hidden = np.asarray(hidden)
    trg_embs = np.asarray(trg_embs)
    vocab_dists = np.asarray(vocab_dists)
    attn_dists = np.asarray(attn_dists)
    src_ids = np.asarray(src_ids)

    in_maps = []
    for i in range(N_CORES):
        bs = slice(i * BPC, (i + 1) * BPC)
        in_maps.append(make_core_inputs(
            context_vecs[bs].reshape(R_FULL, H),
            hidden[bs].reshape(R_FULL, H),
            trg_embs[bs].reshape(R_FULL, H),
            vocab_dists[bs].reshape(R_FULL, V),
            attn_dists[bs].reshape(R_FULL, S),
            src_ids[bs].reshape(R_FULL, S),
            w_h, w_s, w_x_w, w_x_b,
        ))
    return in_maps


def kernel(context_vecs, hidden, trg_embs, vocab_dists, attn_dists,
           src_ids, pad_id, w_h, w_s, w_x_w, w_x_b):
    """Full-input entry point. Shards over 8 NeuronCores, returns [B,T,V] f32."""
    from concourse.bass_utils import run_bass_kernel_spmd

    nc = build_program()
    in_maps = make_in_maps(context_vecs, hidden, trg_embs, vocab_dists,
                           attn_dists, src_ids, w_h, w_s, w_x_w, w_x_b)
    res = run_bass_kernel_spmd(nc, in_maps, list(range(N_CORES)))
    outs = [np.asarray(res.results[i]["out"]).reshape(BPC, T, V)
            for i in range(N_CORES)]
    return np.concatenate(outs, axis=0)



# revision 12
# speedup vs baseline: 1.0217x; 1.0217x over previous
"""CopyMechanism (pointer-generator) kernel for 8 Trainium2 NeuronCores.

Full problem: B=16, T=128, H=512, V=32000, S=400.
  gen = sigmoid(ctx@wh + hid@ws + trg@wx + b)          [B,T,1]
  out = gen * vocab_dists; out[b,t,ids[b,t,s]] += (1-gen)*attn[b,t,s]

Sharding: data-parallel over batch. Core i handles batches [2i, 2i+1]
(256 rows of T-steps). Weights replicated. No cross-core communication.

Device algorithm (per core, per row r):
  Decompose vocab index v = p*250 + f  (V = 128*250), so a row's 32000-wide
  output is an SBUF tile [128 partitions, 250 free].  Using
     out = pg * (vocab + scatter(ratio * attn)),   ratio = (1-pg)/pg,
  the whole row is accumulated in PSUM and scaled once on the way out:
   - base: one fp32r matmul per ROW PAIR with lhsT = I (identity) and
     rhs = vocab[j:j+2] (free 500 >= 256 keeps fp32r at full rate) starts
     the PSUM accumulation group with the raw vocab rows;
   - scatter: per row, 4 bf16 matmuls contract s-chunks of 128:
       M[p,f] += sum_s (onehot(pi[s])[p]*rval[s]) * onehot(fi[s])[f]
     with A = (iotaP==pi)*rval built by tensor_scalar (engine per-chunk
     configurable: VectorE / GpSimd) and B = (iotaF==fi) on VectorE;
   - merge: ScalarE copies PSUM->SBUF with scale = pg (per-partition AP),
     which applies pg to the base and (1-pg) to the scatter in one pass.
  Indices are pre-decomposed and pre-transposed on host (integer-only
  preprocessing).  p_gen is computed on-device (dot products + sigmoid),
  bounced through a DRAM scratch and re-loaded with a partition-broadcast
  AP so each row's scalar is available on all 128 partitions.  Vocab loads
  ride the SP HWDGE ring, out stores the ACT ring.

  Memory-regime choices: vocab is shipped to the device in bf16 and in the
  [p, r, f] layout (host-side cast+transpose, pure data movement), and the
  output leaves the device in bf16 [p, r, f] (host casts back to f32 and
  untransposes).  That halves the dominant vocab+out HBM traffic and makes
  every DMA descriptor a contiguous multi-KB run per partition.
"""

import numpy as np
from ml_dtypes import bfloat16

# ---------------------------------------------------------------------------
# problem constants (hardcoded per contract)
B, T, H, V, S = 16, 128, 512, 32000, 400
N_CORES = 8
BPC = B // N_CORES          # batches per core
R_FULL = BPC * T            # rows per core = 256
FD_FULL = V // 128          # 250
SP_FULL = (S + 127) // 128  # 4 s-chunks
G_FULL = 16                 # rows per vocab DMA group

_PROGRAM_CACHE = {}


def build_program(R=R_FULL, FD=FD_FULL, SP=SP_FULL, G=G_FULL, rep=1,
                  a_eng="vvvv", ablate="full"):
    """Build + compile the per-core Bass program. Same program for all cores.

    R : rows per core (multiple of 128)
    FD: free-dim width of the vocab decomposition (V_local = 128*FD)
    SP: number of 128-wide s-chunks (S padded to SP*128)
    G : rows per vocab/out DMA group (even)
    a_eng: per-chunk engine for the A one-hot build: 'v'=VectorE 'g'=GpSimd
    rep : repeat the whole body rep times (identical output; used for
          differential device-time measurement).
    ablate: "full" | "dmaonly" | "nomm"
    """
    key = (R, FD, SP, G, rep, a_eng, ablate)
    if key in _PROGRAM_CACHE:
        return _PROGRAM_CACHE[key]

    from contextlib import ExitStack

    import concourse.bass as bass
    import concourse.tile as tile
    from concourse import bacc, mybir

    f32 = mybir.dt.float32
    bf16 = mybir.dt.bfloat16
    Alu = mybir.AluOpType
    Act = mybir.ActivationFunctionType
    VL = 128 * FD
    RB = R // 128
    NG = R // G
    assert R % 128 == 0 and R % G == 0 and G % 2 == 0

    nc = bacc.Bacc("TRN2", target_bir_lowering=False, debug=False)

    ctx_d = nc.dram_tensor("ctx", [R, H], f32, kind="ExternalInput")
    hid_d = nc.dram_tensor("hid", [R, H], f32, kind="ExternalInput")
    trg_d = nc.dram_tensor("trg", [R, H], f32, kind="ExternalInput")
    vocab_d = nc.dram_tensor("vocabT", [128, R, FD], bf16, kind="ExternalInput")
    attnT_d = nc.dram_tensor("attnT", [128, RB * SP, 128], f32, kind="ExternalInput")
    piT_d = nc.dram_tensor("piT", [128, RB * SP, 128], f32, kind="ExternalInput")
    fiT_d = nc.dram_tensor("fiT", [128, RB * SP, 128], f32, kind="ExternalInput")
    # weights replicated across partitions on host (pure data movement)
    wh_d = nc.dram_tensor("wh", [128, H], f32, kind="ExternalInput")
    ws_d = nc.dram_tensor("ws", [128, H], f32, kind="ExternalInput")
    wx_d = nc.dram_tensor("wx", [128, H], f32, kind="ExternalInput")
    wxb_d = nc.dram_tensor("wxb", [128, 1], f32, kind="ExternalInput")
    iotaP_d = nc.dram_tensor("iotaP", [128, 128], bf16, kind="ExternalInput")
    iotaF_d = nc.dram_tensor("iotaF", [128, FD], bf16, kind="ExternalInput")
    ident_d = nc.dram_tensor("ident", [128, 128], bf16, kind="ExternalInput")
    out_d = nc.dram_tensor("outT", [128, R, FD], bf16, kind="ExternalOutput")

    with tile.TileContext(nc) as tc, ExitStack() as es:
        singles = es.enter_context(tc.tile_pool(name="singles", bufs=1))
        ph1 = es.enter_context(tc.tile_pool(name="ph1", bufs=2))
        vpool = es.enter_context(tc.tile_pool(name="vpool", bufs=3))
        opool = es.enter_context(tc.tile_pool(name="opool", bufs=3))
        abpool = es.enter_context(tc.tile_pool(name="ab", bufs=8))
        ppool = es.enter_context(tc.tile_pool(name="psum", bufs=8, space="PSUM"))
        dpool = es.enter_context(tc.tile_pool(name="dram", bufs=1, space="DRAM"))

        # --- constants / small inputs ---
        attnT = singles.tile([128, RB * SP, 128], f32)
        nc.sync.dma_start(attnT[:], attnT_d[:])
        piT = singles.tile([128, RB * SP, 128], f32)
        nc.sync.dma_start(piT[:], piT_d[:])
        fiT = singles.tile([128, RB * SP, 128], f32)
        nc.sync.dma_start(fiT[:], fiT_d[:])
        iotaP = singles.tile([128, 128], bf16)
        nc.sync.dma_start(iotaP[:], iotaP_d[:])
        iotaF = singles.tile([128, FD], bf16)
        nc.sync.dma_start(iotaF[:], iotaF_d[:])
        ident = singles.tile([128, 128], bf16)
        nc.sync.dma_start(ident[:], ident_d[:])
        wh = singles.tile([128, H], f32)
        nc.sync.dma_start(wh[:], wh_d[:])
        ws = singles.tile([128, H], f32)
        nc.sync.dma_start(ws[:], ws_d[:])
        wx = singles.tile([128, H], f32)
        nc.sync.dma_start(wx[:], wx_d[:])
        wxb = singles.tile([128, 1], f32)
        nc.sync.dma_start(wxb[:], wxb_d[:])
        scaledT = singles.tile([128, RB * SP, 128], f32)
        pgen_all = singles.tile([128, R], f32)
        rinv_all = singles.tile([128, R], f32)
        ratio_all = singles.tile([128, R], f32)
        pgen_dram = dpool.tile([R, 1], f32)

        # --- phase 1a: p_gen per row (rows on partitions), bounce to DRAM ---
        def _phase1a():
          for blk in range(RB):
            rows = slice(blk * 128, (blk + 1) * 128)
            gacc = ph1.tile([128, 1], f32, tag="gacc")
            gtmp = ph1.tile([128, 1], f32, tag="gtmp")
            g2 = ph1.tile([128, 1], f32, tag="g2")
            prod = ph1.tile([128, H], f32, tag="prod")
            for i, (src_d, w) in enumerate(
                ((ctx_d, wh), (hid_d, ws), (trg_d, wx))
            ):
                x = ph1.tile([128, H], f32, tag="x")
                nc.sync.dma_start(x[:], src_d[rows, :])
                nc.vector.tensor_tensor(prod[:], x[:], w[:], op=Alu.mult)
                dst = (gacc, gtmp, g2)[i]
                nc.vector.tensor_reduce(
                    dst[:], prod[:], axis=mybir.AxisListType.X, op=Alu.add
                )
            gsum = ph1.tile([128, 1], f32, tag="gsum")
            nc.vector.tensor_tensor(gsum[:], gacc[:], gtmp[:], op=Alu.add)
            gall = ph1.tile([128, 1], f32, tag="gall")
            nc.vector.tensor_tensor(gall[:], gsum[:], g2[:], op=Alu.add)
            pgen_col = ph1.tile([128, 1], f32, tag="pgen")
            nc.scalar.activation(
                pgen_col[:], gall[:], Act.Sigmoid, bias=wxb[:], scale=1.0
            )
            nc.sync.dma_start(pgen_dram[rows, :], pgen_col[:])

        # --- phase 1b: broadcast p_gen; ratio = (1-pg)/pg; scaled attnT ---
        def _phase1b():
            pg_flat = pgen_dram[:, 0]
            pg_bcast = bass.AP(
                tensor=pg_flat.tensor, offset=pg_flat.offset,
                ap=[[0, 128]] + list(pg_flat.ap),
            )
            nc.gpsimd.dma_start(pgen_all[:], pg_bcast)
            nc.vector.reciprocal(rinv_all[:], pgen_all[:])
            # ratio = (1 - pg) / pg = 1/pg - 1
            nc.vector.tensor_scalar(
                ratio_all[:], rinv_all[:], 1.0, None, Alu.subtract
            )
            for blk in range(RB):
                for c in range(SP):
                    nc.vector.tensor_tensor(
                        scaledT[:, blk * SP + c, :],
                        attnT[:, blk * SP + c, :],
                        ratio_all[:, blk * 128:(blk + 1) * 128],
                        op=Alu.mult,
                    )

        # --- phase 2: base via bf16 identity matmul + scatter matmuls ---
        def _phase2():
          for grp in range(NG):
            gr = slice(grp * G, (grp + 1) * G)
            vt = vpool.tile([128, G, FD], bf16)
            nc.sync.dma_start(vt[:], vocab_d[:, gr, :])
            ot = opool.tile([128, G, FD], bf16)
            if ablate == "dmaonly":
                nc.scalar.copy(ot[:, :, :], vt[:, :, :])
                nc.scalar.dma_start(out_d[:, gr, :], ot[:])
                continue
            for j in range(0, G, 2):
                psb = ppool.tile([128, 2, FD], f32)
                if ablate != "nomm":
                    nc.tensor.matmul(
                        psb[:, :, :], lhsT=ident[:],
                        rhs=vt[:, j:j + 2, :],
                        start=True, stop=False,
                    )
                for jj in range(2):
                    r = grp * G + j + jj
                    blk = r // 128
                    rl = r % 128
                    pg_sc = pgen_all[:, r:r + 1]
                    ps = psb[:, jj, :]
                    for c in range(SP):
                        ch = blk * SP + c
                        # A carries the value: A[s,p] = (pi[s]==p) * rval[s]
                        A = abpool.tile([128, 128], bf16, tag="A")
                        eng = nc.gpsimd if a_eng[c] == "g" else nc.vector
                        eng.tensor_scalar(
                            A[:], iotaP[:], piT[:, ch, rl:rl + 1],
                            scaledT[:, ch, rl:rl + 1], Alu.is_equal, Alu.mult,
                        )
                        # B is the pure one-hot of fi (1-op, wide)
                        Bt = abpool.tile([128, FD], bf16, tag="B")
                        nc.vector.tensor_scalar(
                            Bt[:], iotaF[:], fiT[:, ch, rl:rl + 1], None,
                            Alu.is_equal,
                        )
                        if ablate == "nomm":
                            continue
                        nc.tensor.matmul(
                            ps, lhsT=A[:], rhs=Bt[:],
                            start=False,
                            stop=(jj == 1 and c == SP - 1),
                        )
                    if ablate == "nomm":
                        nc.scalar.mul(ot[:, j + jj, :], vt[:, j + jj, :], pg_sc)
                    else:
                        # PSUM -> SBUF with the p_gen scale applied in-flight
                        nc.scalar.mul(ot[:, j + jj, :], ps, pg_sc)
            nc.scalar.dma_start(out_d[:, gr, :], ot[:])

        for _ in range(rep):
            _phase1a()
            _phase1b()
            _phase2()

    nc.compile()
    _PROGRAM_CACHE[key] = nc
    return nc


def make_core_inputs(ctx, hid, trg, vocab, attn, ids, w_h, w_s, w_x_w, w_x_b,
                     R=R_FULL, FD=FD_FULL, SP=SP_FULL):
    """Host-side prep for one core: flatten rows, decompose + transpose indices.

    ctx/hid/trg: [R, H] f32; vocab: [R, 128*FD] f32; attn: [R, S'] f32;
    ids: [R, S'] int. Returns the in_map dict for this core.
    """
    RB = R // 128
    Sp = SP * 128
    Sl = attn.shape[1]
    f32 = np.float32

    ids = np.asarray(ids).astype(np.int64)
    pi = (ids // FD).astype(f32)
    fi = (ids % FD).astype(f32)

    def tr(x, pad):
        full = np.full((R, Sp), pad, dtype=f32)
        full[:, :Sl] = x
        # [R, Sp] -> [RB, 128(r), SP, 128(s)] -> [s, RB, SP, r]
        t = full.reshape(RB, 128, SP, 128).transpose(3, 0, 2, 1)
        return np.ascontiguousarray(t.reshape(128, RB * SP, 128))

    def rep(w, n):
        return np.ascontiguousarray(
            np.broadcast_to(np.asarray(w, dtype=f32).reshape(1, n), (128, n))
        )

    # vocab: cast to bf16 and transpose to [p, r, f] so device DMAs are
    # contiguous multi-KB runs per partition (pure data movement + rounding)
    vocabT = np.ascontiguousarray(
        np.asarray(vocab, dtype=f32).astype(bfloat16)
        .reshape(R, 128, FD).transpose(1, 0, 2)
    )

    return {
        "ctx": np.ascontiguousarray(ctx, dtype=f32),
        "hid": np.ascontiguousarray(hid, dtype=f32),
        "trg": np.ascontiguousarray(trg, dtype=f32),
        "vocabT": vocabT,
        "attnT": tr(np.asarray(attn, dtype=f32), 0.0),
        "piT": tr(pi, 1.0e4),
        "fiT": tr(fi, -1.0),
        "wh": rep(w_h, H),
        "ws": rep(w_s, H),
        "wx": rep(w_x_w, H),
        "wxb": rep(w_x_b, 1),
        "iotaP": rep(np.arange(128, dtype=f32), 128).astype(bfloat16),
        "iotaF": rep(np.arange(FD, dtype=f32), FD).astype(bfloat16),
        "ident": np.eye(128, dtype=np.float32).astype(bfloat16),
    }


def make_in_maps(context_vecs, hidden, trg_embs, vocab_dists, attn_dists,
                 src_ids, w_h, w_s, w_x_w, w_x_b):
    """Build the 8 per-core input dicts from full inputs."""
    context_vecs = np.asarray(context_vecs)
    hidden = np.asarray(hidden)
    trg_embs = np.asarray(trg_embs)
    vocab_dists = np.asarray(vocab_dists)
    attn_dists = np.asarray(attn_dists)
    src_ids = np.asarray(src_ids)

    in_maps = []
    for i in range(N_CORES):
        bs = slice(i * BPC, (i + 1) * BPC)
        in_maps.append(make_core_inputs(
            context_vecs[bs].reshape(R_FULL, H),
            hidden[bs].reshape(R_FULL, H),
            trg_embs[bs].reshape(R_FULL, H),
            vocab_dists[bs].reshape(R_FULL, V),
            attn_dists[bs].reshape(R_FULL, S),
            src_ids[bs].reshape(R_FULL, S),
            w_h, w_s, w_x_w, w_x_b,
        ))
    return in_maps


def kernel(context_vecs, hidden, trg_embs, vocab_dists, attn_dists,
           src_ids, pad_id, w_h, w_s, w_x_w, w_x_b):
    """Full-input entry point. Shards over 8 NeuronCores, returns [B,T,V] f32."""
    from concourse.bass_utils import run_bass_kernel_spmd

    nc = build_program()
    in_maps = make_in_maps(context_vecs, hidden, trg_embs, vocab_dists,
                           attn_dists, src_ids, w_h, w_s, w_x_w, w_x_b)
    res = run_bass_kernel_spmd(nc, in_maps, list(range(N_CORES)))
    outs = []
    for i in range(N_CORES):
        # [128, R, FD] bf16 -> [R, 128*FD] f32
        o = np.asarray(res.results[i]["outT"]).astype(np.float32)
        outs.append(o.transpose(1, 0, 2).reshape(BPC, T, V))
    return np.concatenate(outs, axis=0)


# revision 24
# speedup vs baseline: 23.4404x; 22.9422x over previous
"""CopyMechanism (pointer-generator) kernel for 8 Trainium2 NeuronCores.

Full problem: B=16, T=128, H=512, V=32000, S=400.
  gen = sigmoid(ctx@wh + hid@ws + trg@wx + b)          [B,T,1]
  out = gen * vocab_dists; out[b,t,ids[b,t,s]] += (1-gen)*attn[b,t,s]

Sharding: data-parallel over batch. Core i handles batches [2i, 2i+1]
(256 rows of T-steps). Weights replicated. No cross-core communication.

Device algorithm (per core, per row r):
  Decompose vocab index v = p*250 + f  (V = 128*250), so a row's 32000-wide
  output is an SBUF tile [128 partitions, 250 free].  Using
     out = pg * (vocab + scatter(ratio * attn)),   ratio = (1-pg)/pg,
  the whole row is accumulated in PSUM and scaled once on the way out:
   - base: one bf16 matmul per ROW PAIR with lhsT = I (identity) and
     rhs = vocab[j:j+2] starts the PSUM accumulation group with the raw
     vocab rows;
   - scatter: per row, 3 bf16 matmuls contract s-chunks of 128 (s<384):
       M[p,f] += sum_s (onehot(pi[s])[p]*rval[s]) * onehot(fi[s])[f]
     with A = (iotaP==pi)*rval and B = (iotaF==fi) built by VectorE
     tensor_scalar ops in bf16 (4x perf mode).  The 16 leftover entries
     (s in [384,400)) of BOTH pair rows are packed into one 32-contraction
     matmul whose B' one-hot spans the pair's 500-wide PSUM block
     (fi' = fi + 250*row_parity) — 25% fewer one-hot builds than 4 chunks.
   - merge: ScalarE copies PSUM->SBUF with scale = pg (per-partition AP),
     which applies pg to the base and (1-pg) to the scatter in one pass.
  p_gen is computed on the PE (12 thin matmuls against host-transposed
  activations), sigmoid on ScalarE, bounced through a DRAM scratch and
  re-loaded with a partition-broadcast AP so each row's scalar reaches all
  128 partitions.  Indices are pre-decomposed/transposed on host
  (integer-only preprocessing).

  Memory-regime choices: vocab is shipped to the device in bf16 and in the
  [p, r, f] layout (host-side cast+transpose, pure data movement), and the
  output leaves the device in bf16 [p, r, f] (host casts back to f32 and
  untransposes).  That halves the dominant vocab+out HBM traffic and makes
  every DMA descriptor a contiguous multi-KB run per partition.  Vocab
  loads ride the SP HWDGE ring, out stores the ACT ring.
"""

import numpy as np
from ml_dtypes import bfloat16

# ---------------------------------------------------------------------------
# problem constants (hardcoded per contract)
B, T, H, V, S = 16, 128, 512, 32000, 400
N_CORES = 8
BPC = B // N_CORES          # batches per core
R_FULL = BPC * T            # rows per core = 256
FD_FULL = V // 128          # 250
SP_FULL = 3                 # full 128-wide s-chunks (s < 384)
SL = S - 128 * SP_FULL      # leftover entries per row = 16
G_FULL = 16                 # rows per vocab DMA group

_PROGRAM_CACHE = {}


def build_program(R=R_FULL, FD=FD_FULL, SP=SP_FULL, G=G_FULL, rep=1,
                  ablate="full"):
    """Build + compile the per-core Bass program. Same program for all cores.

    R : rows per core (multiple of 128)
    FD: free-dim width of the vocab decomposition (V_local = 128*FD)
    SP: number of full 128-wide s-chunks
    G : rows per vocab/out DMA group (even)
    rep : repeat the whole body rep times (identical output; used for
          differential device-time measurement).
    ablate: "full" | "dmaonly" | "nomm"
    """
    key = (R, FD, SP, G, rep, ablate)
    if key in _PROGRAM_CACHE:
        return _PROGRAM_CACHE[key]

    from contextlib import ExitStack

    import concourse.bass as bass
    import concourse.tile as tile
    from concourse import bacc, mybir

    f32 = mybir.dt.float32
    bf16 = mybir.dt.bfloat16
    Alu = mybir.AluOpType
    Act = mybir.ActivationFunctionType
    RB = R // 128
    NG = R // G
    NPAIR = R // 2
    HB = H // 128
    assert R % 128 == 0 and R % G == 0 and G % 2 == 0

    nc = bacc.Bacc("TRN2", target_bir_lowering=False, debug=False)

    # host-transposed activations [H, R] for the PE-side p_gen dot products
    ctxT_d = nc.dram_tensor("ctxT", [H, R], f32, kind="ExternalInput")
    hidT_d = nc.dram_tensor("hidT", [H, R], f32, kind="ExternalInput")
    trgT_d = nc.dram_tensor("trgT", [H, R], f32, kind="ExternalInput")
    vocab_d = nc.dram_tensor("vocabT", [128, R, FD], bf16, kind="ExternalInput")
    attnT_d = nc.dram_tensor("attnT", [128, RB * SP, 128], f32, kind="ExternalInput")
    piT_d = nc.dram_tensor("piT", [128, RB * SP, 128], f32, kind="ExternalInput")
    fiT_d = nc.dram_tensor("fiT", [128, RB * SP, 128], f32, kind="ExternalInput")
    # pair-packed leftover entries (s in [384,400) of both pair rows);
    # even row at partitions 0:16, odd row at 32:48 (32-aligned slices)
    SL2 = 64
    attnT2_d = nc.dram_tensor("attnT2", [SL2, NPAIR], f32, kind="ExternalInput")
    piT2_d = nc.dram_tensor("piT2", [SL2, NPAIR], f32, kind="ExternalInput")
    fiT2_d = nc.dram_tensor("fiT2", [SL2, NPAIR], f32, kind="ExternalInput")
    # weights in [hl, c] chunk layout for the PE dot products
    whT_d = nc.dram_tensor("whT", [128, HB], f32, kind="ExternalInput")
    wsT_d = nc.dram_tensor("wsT", [128, HB], f32, kind="ExternalInput")
    wxT_d = nc.dram_tensor("wxT", [128, HB], f32, kind="ExternalInput")
    wxb_d = nc.dram_tensor("wxb", [128, 1], f32, kind="ExternalInput")
    iotaP_d = nc.dram_tensor("iotaP", [128, 128], bf16, kind="ExternalInput")
    iotaF_d = nc.dram_tensor("iotaF", [128, FD], bf16, kind="ExternalInput")
    # f32: bf16 can't represent odd integers above 256 exactly
    iotaF2_d = nc.dram_tensor("iotaF2", [128, 2 * FD], f32, kind="ExternalInput")
    ident_d = nc.dram_tensor("ident", [128, 128], bf16, kind="ExternalInput")
    out_d = nc.dram_tensor("outT", [128, R, FD], bf16, kind="ExternalOutput")

    with tile.TileContext(nc) as tc, ExitStack() as es:
        singles = es.enter_context(tc.tile_pool(name="singles", bufs=1))
        ph1 = es.enter_context(tc.tile_pool(name="ph1", bufs=2))
        vpool = es.enter_context(tc.tile_pool(name="vpool", bufs=4))
        opool = es.enter_context(tc.tile_pool(name="opool", bufs=4))
        abpool = es.enter_context(tc.tile_pool(name="ab", bufs=24))
        ppool = es.enter_context(tc.tile_pool(name="psum", bufs=7, space="PSUM"))
        pp1 = es.enter_context(tc.tile_pool(name="psum1", bufs=1, space="PSUM"))
        dpool = es.enter_context(tc.tile_pool(name="dram", bufs=1, space="DRAM"))

        # --- constants / small inputs ---
        attnT = singles.tile([128, RB * SP, 128], f32)
        nc.sync.dma_start(attnT[:], attnT_d[:])
        piT = singles.tile([128, RB * SP, 128], f32)
        nc.sync.dma_start(piT[:], piT_d[:])
        fiT = singles.tile([128, RB * SP, 128], f32)
        nc.sync.dma_start(fiT[:], fiT_d[:])
        attnT2 = singles.tile([SL2, NPAIR], f32)
        nc.sync.dma_start(attnT2[:], attnT2_d[:])
        piT2 = singles.tile([SL2, NPAIR], f32)
        nc.sync.dma_start(piT2[:], piT2_d[:])
        fiT2 = singles.tile([SL2, NPAIR], f32)
        nc.sync.dma_start(fiT2[:], fiT2_d[:])
        iotaP = singles.tile([128, 128], bf16)
        nc.sync.dma_start(iotaP[:], iotaP_d[:])
        iotaF = singles.tile([128, FD], bf16)
        nc.sync.dma_start(iotaF[:], iotaF_d[:])
        iotaF2 = singles.tile([128, 2 * FD], f32)
        nc.sync.dma_start(iotaF2[:], iotaF2_d[:])
        ident = singles.tile([128, 128], bf16)
        nc.sync.dma_start(ident[:], ident_d[:])
        whT = singles.tile([128, HB], f32)
        nc.sync.dma_start(whT[:], whT_d[:])
        wsT = singles.tile([128, HB], f32)
        nc.sync.dma_start(wsT[:], wsT_d[:])
        wxT = singles.tile([128, HB], f32)
        nc.sync.dma_start(wxT[:], wxT_d[:])
        wxb = singles.tile([128, 1], f32)
        nc.sync.dma_start(wxb[:], wxb_d[:])
        scaledT = singles.tile([128, RB * SP, 128], f32)
        scaledT2 = singles.tile([SL2, NPAIR], f32)
        pgen_all = singles.tile([128, R], f32)
        rinv_all = singles.tile([128, R], f32)
        ratio_all = singles.tile([128, R], f32)
        pgen_dram = dpool.tile([1, R], f32)

        # --- phase 1a: p_gen per row on the PE, bounce to DRAM ---
        def _phase1a():
            xs = []
            for nm, src_d in (("c", ctxT_d), ("h", hidT_d), ("t", trgT_d)):
                xT = ph1.tile([128, HB, R], f32, tag=f"x{nm}")
                nc.sync.dma_start(
                    xT[:], src_d[:].rearrange("(c p) r -> p c r", p=128)
                )
                xs.append(xT)
            gps = pp1.tile([1, R], f32)
            for i, (xT, wT) in enumerate(zip(xs, (whT, wsT, wxT))):
                for c in range(HB):
                    nc.tensor.matmul(
                        gps[0:1, :], lhsT=wT[:, c:c + 1], rhs=xT[:, c, :],
                        start=(i == 0 and c == 0),
                        stop=(i == 2 and c == HB - 1),
                    )
            pgrow = ph1.tile([1, R], f32, tag="pgrow")
            nc.scalar.activation(
                pgrow[0:1, :], gps[0:1, :], Act.Sigmoid,
                bias=wxb[0:1, :], scale=1.0,
            )
            nc.sync.dma_start(pgen_dram[:], pgrow[0:1, :])

        # --- phase 1b: broadcast p_gen; ratio = (1-pg)/pg; scaled attn ---
        def _phase1b():
            nc.gpsimd.dma_start(
                pgen_all[:], pgen_dram[0, :].partition_broadcast(128)
            )
            nc.vector.reciprocal(rinv_all[:], pgen_all[:])
            # ratio = (1 - pg) / pg = 1/pg - 1
            nc.vector.tensor_scalar(
                ratio_all[:], rinv_all[:], 1.0, None, Alu.subtract
            )
            for blk in range(RB):
                for c in range(SP):
                    nc.vector.tensor_tensor(
                        scaledT[:, blk * SP + c, :],
                        attnT[:, blk * SP + c, :],
                        ratio_all[:, blk * 128:(blk + 1) * 128],
                        op=Alu.mult,
                    )
            # leftover entries: partitions 0:16 hold the even row (its ratio
            # in even columns), 32:48 the odd row; unused rows have attn=0 so
            # multiplying them by a garbage ratio still yields 0
            nc.vector.tensor_tensor(
                scaledT2[0:32, :], attnT2[0:32, :],
                ratio_all[0:32, 0:R:2], op=Alu.mult,
            )
            nc.vector.tensor_tensor(
                scaledT2[32:64, :], attnT2[32:64, :],
                ratio_all[32:64, 1:R:2], op=Alu.mult,
            )

        # --- phase 2: base via bf16 identity matmul + scatter matmuls ---
        def _phase2():
          for grp in range(NG):
            gr = slice(grp * G, (grp + 1) * G)
            vt = vpool.tile([128, G, FD], bf16)
            nc.sync.dma_start(vt[:], vocab_d[:, gr, :])
            ot = opool.tile([128, G, FD], bf16)
            if ablate == "dmaonly":
                nc.scalar.copy(ot[:, :, :], vt[:, :, :])
                nc.scalar.dma_start(out_d[:, gr, :], ot[:])
                continue
            for j in range(0, G, 2):
                pr = (grp * G + j) // 2
                psb = ppool.tile([128, 2, FD], f32)
                if ablate != "nomm":
                    nc.tensor.matmul(
                        psb[:, :, :], lhsT=ident[:],
                        rhs=vt[:, j:j + 2, :],
                        start=True, stop=False,
                    )
                for jj in range(2):
                    r = grp * G + j + jj
                    blk = r // 128
                    rl = r % 128
                    ps = psb[:, jj, :]
                    for c in range(SP):
                        ch = blk * SP + c
                        # A carries the value: A[s,p] = (pi[s]==p) * rval[s]
                        A = abpool.tile([128, 128], bf16, tag="A")
                        nc.vector.tensor_scalar(
                            A[:], iotaP[:], piT[:, ch, rl:rl + 1],
                            scaledT[:, ch, rl:rl + 1], Alu.is_equal, Alu.mult,
                        )
                        # B is the pure one-hot of fi (1-op, wide)
                        Bt = abpool.tile([128, FD], bf16, tag="B")
                        nc.vector.tensor_scalar(
                            Bt[:], iotaF[:], fiT[:, ch, rl:rl + 1], None,
                            Alu.is_equal,
                        )
                        if ablate == "nomm":
                            continue
                        nc.tensor.matmul(
                            ps, lhsT=A[:], rhs=Bt[:],
                            start=False, stop=False,
                        )
                # pair-packed leftover chunk: 32 entries scatter into the
                # pair's full 500-wide PSUM block (fi' = fi + 250*parity)
                A2 = abpool.tile([SL2, 128], bf16, tag="A2")
                nc.vector.tensor_scalar(
                    A2[:], iotaP[0:SL2, :], piT2[:, pr:pr + 1],
                    scaledT2[:, pr:pr + 1], Alu.is_equal, Alu.mult,
                )
                B2 = abpool.tile([SL2, 2 * FD], bf16, tag="B2")
                nc.vector.tensor_scalar(
                    B2[:], iotaF2[0:SL2, :], fiT2[:, pr:pr + 1], None,
                    Alu.is_equal,
                )
                if ablate != "nomm":
                    nc.tensor.matmul(
                        psb[:, :, :], lhsT=A2[:], rhs=B2[:],
                        start=False, stop=True,
                    )
                for jj in range(2):
                    r = grp * G + j + jj
                    pg_sc = pgen_all[:, r:r + 1]
                    if ablate == "nomm":
                        nc.scalar.mul(ot[:, j + jj, :], vt[:, j + jj, :], pg_sc)
                    else:
                        # PSUM -> SBUF with the p_gen scale applied in-flight
                        nc.scalar.mul(ot[:, j + jj, :], psb[:, jj, :], pg_sc)
            nc.scalar.dma_start(out_d[:, gr, :], ot[:])

        for _ in range(rep):
            _phase1a()
            _phase1b()
            _phase2()

    nc.compile()
    _PROGRAM_CACHE[key] = nc
    return nc


def make_core_inputs(ctx, hid, trg, vocab, attn, ids, w_h, w_s, w_x_w, w_x_b,
                     R=R_FULL, FD=FD_FULL, SP=SP_FULL):
    """Host-side prep for one core: flatten rows, decompose + transpose indices.

    ctx/hid/trg: [R, H] f32; vocab: [R, 128*FD] f32; attn: [R, S'] f32;
    ids: [R, S'] int. Returns the in_map dict for this core.
    """
    RB = R // 128
    Smain = SP * 128
    f32 = np.float32

    ids = np.asarray(ids).astype(np.int64)
    pi = (ids // FD).astype(f32)
    fi = (ids % FD).astype(f32)
    attn = np.asarray(attn, dtype=f32)

    def tr(x):
        # [R, Smain] -> [RB, 128(r), SP, 128(s)] -> [s, RB, SP, r]
        t = (np.ascontiguousarray(x[:, :Smain])
             .reshape(RB, 128, SP, 128).transpose(3, 0, 2, 1))
        return np.ascontiguousarray(t.reshape(128, RB * SP, 128))

    def tr2(x, fill=0.0):
        # leftover entries of pair rows: [R, SL] -> [64, R/2] with the even
        # row's entries at partitions 0:16 and the odd row's at 32:48
        # (32-aligned partition slices on device); unused rows get `fill`
        out = np.full((64, R // 2), fill, dtype=f32)
        pair = x[:, Smain:S].reshape(R // 2, 2, SL)
        out[0:SL, :] = pair[:, 0, :].T
        out[32:32 + SL, :] = pair[:, 1, :].T
        return np.ascontiguousarray(out)

    def rep(w, n):
        return np.ascontiguousarray(
            np.broadcast_to(np.asarray(w, dtype=f32).reshape(1, n), (128, n))
        )

    fiT2 = tr2(fi, fill=-1.0)
    fiT2[32:32 + SL, :] += FD  # odd row scatters into the upper 250 columns

    # vocab: cast to bf16 and transpose to [p, r, f] so device DMAs are
    # contiguous multi-KB runs per partition (pure data movement + rounding)
    vocabT = np.ascontiguousarray(
        np.asarray(vocab, dtype=f32).astype(bfloat16)
        .reshape(R, 128, FD).transpose(1, 0, 2)
    )

    def wchunks(w):
        # [H] -> [128, H/128]: column c holds weights for h in [128c,128c+128)
        return np.ascontiguousarray(
            np.asarray(w, dtype=f32).reshape(-1).reshape(H // 128, 128).T
        )

    return {
        "ctxT": np.ascontiguousarray(np.asarray(ctx, dtype=f32).T),
        "hidT": np.ascontiguousarray(np.asarray(hid, dtype=f32).T),
        "trgT": np.ascontiguousarray(np.asarray(trg, dtype=f32).T),
        "vocabT": vocabT,
        "attnT": tr(attn),
        "piT": tr(pi),
        "fiT": tr(fi),
        "attnT2": tr2(attn),
        "piT2": tr2(pi, fill=-1.0),
        "fiT2": np.ascontiguousarray(fiT2),
        "whT": wchunks(w_h),
        "wsT": wchunks(w_s),
        "wxT": wchunks(w_x_w),
        "wxb": rep(w_x_b, 1),
        "iotaP": rep(np.arange(128, dtype=f32), 128).astype(bfloat16),
        "iotaF": rep(np.arange(FD, dtype=f32), FD).astype(bfloat16),
        "iotaF2": rep(np.arange(2 * FD, dtype=f32), 2 * FD),
        "ident": np.eye(128, dtype=np.float32).astype(bfloat16),
    }


def make_in_maps(context_vecs, hidden, trg_embs, vocab_dists, attn_dists,
                 src_ids, w_h, w_s, w_x_w, w_x_b):
    """Build the 8 per-core input dicts from full inputs."""
    context_vecs = np.asarray(context_vecs)
    hidden = np.asarray(hidden)
    trg_embs = np.asarray(trg_embs)
    vocab_dists = np.asarray(vocab_dists)
    attn_dists = np.asarray(attn_dists)
    src_ids = np.asarray(src_ids)

    in_maps = []
    for i in range(N_CORES):
        bs = slice(i * BPC, (i + 1) * BPC)
        in_maps.append(make_core_inputs(
            context_vecs[bs].reshape(R_FULL, H),
            hidden[bs].reshape(R_FULL, H),
            trg_embs[bs].reshape(R_FULL, H),
            vocab_dists[bs].reshape(R_FULL, V),
            attn_dists[bs].reshape(R_FULL, S),
            src_ids[bs].reshape(R_FULL, S),
            w_h, w_s, w_x_w, w_x_b,
        ))
    return in_maps


def kernel(context_vecs, hidden, trg_embs, vocab_dists, attn_dists,
           src_ids, pad_id, w_h, w_s, w_x_w, w_x_b):
    """Full-input entry point. Shards over 8 NeuronCores, returns [B,T,V] f32."""
    from concourse.bass_utils import run_bass_kernel_spmd

    nc = build_program()
    in_maps = make_in_maps(context_vecs, hidden, trg_embs, vocab_dists,
                           attn_dists, src_ids, w_h, w_s, w_x_w, w_x_b)
    res = run_bass_kernel_spmd(nc, in_maps, list(range(N_CORES)))
    outs = []
    for i in range(N_CORES):
        # [128, R, FD] bf16 -> [R, 128*FD] f32
        o = np.asarray(res.results[i]["outT"]).astype(np.float32)
        outs.append(o.transpose(1, 0, 2).reshape(BPC, T, V))
    return np.concatenate(outs, axis=0)


# revision 36
# speedup vs baseline: 26.7581x; 1.1415x over previous
"""CopyMechanism (pointer-generator) kernel for 8 Trainium2 NeuronCores.

Full problem: B=16, T=128, H=512, V=32000, S=400.
  gen = sigmoid(ctx@wh + hid@ws + trg@wx + b)          [B,T,1]
  out = gen * vocab_dists; out[b,t,ids[b,t,s]] += (1-gen)*attn[b,t,s]

Sharding: data-parallel over batch. Core i handles batches [2i, 2i+1]
(256 rows of T-steps). Weights replicated. No cross-core communication.

Device algorithm (per core, per row r):
  Decompose vocab index v = p*250 + f  (V = 128*250), so a row's 32000-wide
  output is an SBUF tile [128 partitions, 250 free].  Using
     out = pg * (vocab + scatter(ratio * attn)),   ratio = (1-pg)/pg,
  the whole row is accumulated in PSUM and scaled once on the way out:
   - base: one bf16 matmul per ROW PAIR with lhsT = I (identity) and
     rhs = vocab[j:j+2] starts the PSUM accumulation group with the raw
     vocab rows;
   - scatter: per row, 3 bf16 matmuls contract s-chunks of 128 (s<384):
       M[p,f] += sum_s (onehot(pi[s])[p]*rval[s]) * onehot(fi[s])[f]
     with A = (iotaP==pi)*rval and B = (iotaF==fi) built by VectorE
     tensor_scalar ops in bf16 (4x perf mode).  The 16 leftover entries
     (s in [384,400)) of BOTH pair rows are packed into one 32-contraction
     matmul whose B' one-hot spans the pair's 500-wide PSUM block
     (fi' = fi + 250*row_parity) — 25% fewer one-hot builds than 4 chunks.
   - merge: ScalarE copies PSUM->SBUF with scale = pg (per-partition AP),
     which applies pg to the base and (1-pg) to the scatter in one pass.
  p_gen is computed on the PE (12 thin matmuls against host-transposed
  activations), sigmoid on ScalarE, bounced through a DRAM scratch and
  re-loaded with a partition-broadcast AP so each row's scalar reaches all
  128 partitions.  Indices are pre-decomposed/transposed on host
  (integer-only preprocessing).

  Memory-regime choices: vocab is shipped to the device in bf16 and in the
  [p, r, f] layout (host-side cast+transpose, pure data movement), and the
  output leaves the device in bf16 [p, r, f] (host casts back to f32 and
  untransposes).  That halves the dominant vocab+out HBM traffic and makes
  every DMA descriptor a contiguous multi-KB run per partition.  Vocab
  loads ride the SP HWDGE ring, out stores the ACT ring.
"""

import numpy as np
from ml_dtypes import bfloat16

# ---------------------------------------------------------------------------
# problem constants (hardcoded per contract)
B, T, H, V, S = 16, 128, 512, 32000, 400
N_CORES = 8
BPC = B // N_CORES          # batches per core
R_FULL = BPC * T            # rows per core = 256
FD_FULL = V // 128          # 250
SP_FULL = 3                 # full 128-wide s-chunks (s < 384)
SL = S - 128 * SP_FULL      # leftover entries per row = 16
G_FULL = 16                 # rows per vocab DMA group

_PROGRAM_CACHE = {}


def build_program(R=R_FULL, FD=FD_FULL, SP=SP_FULL, G=G_FULL, rep=1,
                  ablate="full", scal16=False):
    """Build + compile the per-core Bass program. Same program for all cores.

    R : rows per core (multiple of 128)
    FD: free-dim width of the vocab decomposition (V_local = 128*FD)
    SP: number of full 128-wide s-chunks
    G : rows per vocab/out DMA group (even)
    rep : repeat the whole body rep times (identical output; used for
          differential device-time measurement).
    ablate: "full" | "dmaonly" | "nomm"
    """
    key = (R, FD, SP, G, rep, ablate, scal16)
    if key in _PROGRAM_CACHE:
        return _PROGRAM_CACHE[key]

    from contextlib import ExitStack

    import concourse.bass as bass
    import concourse.tile as tile
    from concourse import bacc, mybir

    f32 = mybir.dt.float32
    bf16 = mybir.dt.bfloat16
    Alu = mybir.AluOpType
    Act = mybir.ActivationFunctionType
    RB = R // 128
    NG = R // G
    NPAIR = R // 2
    HB = H // 128
    assert R % 128 == 0 and R % G == 0 and G % 2 == 0

    nc = bacc.Bacc("TRN2", target_bir_lowering=False, debug=False)

    # host-transposed activations [H, R] for the PE-side p_gen dot products
    ctxT_d = nc.dram_tensor("ctxT", [H, R], f32, kind="ExternalInput")
    hidT_d = nc.dram_tensor("hidT", [H, R], f32, kind="ExternalInput")
    trgT_d = nc.dram_tensor("trgT", [H, R], f32, kind="ExternalInput")
    vocab_d = nc.dram_tensor("vocabT", [128, R, FD], bf16, kind="ExternalInput")
    attnT_d = nc.dram_tensor("attnT", [128, RB * SP, 128], f32, kind="ExternalInput")
    piT_d = nc.dram_tensor("piT", [128, RB * SP, 128], f32, kind="ExternalInput")
    fiT_d = nc.dram_tensor("fiT", [128, RB * SP, 128], f32, kind="ExternalInput")
    # pair-packed leftover entries (s in [384,400) of both pair rows);
    # even row at partitions 0:16, odd row at 32:48 (32-aligned slices)
    SL2 = 64
    attnT2_d = nc.dram_tensor("attnT2", [SL2, NPAIR], f32, kind="ExternalInput")
    piT2_d = nc.dram_tensor("piT2", [SL2, NPAIR], f32, kind="ExternalInput")
    fiT2_d = nc.dram_tensor("fiT2", [SL2, NPAIR], f32, kind="ExternalInput")
    # weights in [hl, c] chunk layout for the PE dot products
    whT_d = nc.dram_tensor("whT", [128, HB], f32, kind="ExternalInput")
    wsT_d = nc.dram_tensor("wsT", [128, HB], f32, kind="ExternalInput")
    wxT_d = nc.dram_tensor("wxT", [128, HB], f32, kind="ExternalInput")
    wxb_d = nc.dram_tensor("wxb", [128, 1], f32, kind="ExternalInput")
    iotaP_d = nc.dram_tensor("iotaP", [128, 128], bf16, kind="ExternalInput")
    iotaF_d = nc.dram_tensor("iotaF", [128, FD], bf16, kind="ExternalInput")
    # fp16 (not bf16): bf16 can't represent odd integers above 256 exactly,
    # fp16 is exact to 2048 and keeps the 4x DVE perf mode
    f16 = mybir.dt.float16
    iotaF2_d = nc.dram_tensor("iotaF2", [128, 2 * FD], f16, kind="ExternalInput")
    ident_d = nc.dram_tensor("ident", [128, 128], bf16, kind="ExternalInput")
    out_d = nc.dram_tensor("outT", [128, R, FD], bf16, kind="ExternalOutput")

    with tile.TileContext(nc) as tc, ExitStack() as es:
        singles = es.enter_context(tc.tile_pool(name="singles", bufs=1))
        ph1 = es.enter_context(tc.tile_pool(name="ph1", bufs=2))
        vpool = es.enter_context(tc.tile_pool(name="vpool", bufs=4))
        opool = es.enter_context(tc.tile_pool(name="opool", bufs=4))
        abpool = es.enter_context(tc.tile_pool(name="ab", bufs=24))
        ppool = es.enter_context(tc.tile_pool(name="psum", bufs=7, space="PSUM"))
        pp1 = es.enter_context(tc.tile_pool(name="psum1", bufs=1, space="PSUM"))
        dpool = es.enter_context(tc.tile_pool(name="dram", bufs=1, space="DRAM"))

        # --- constants / small inputs ---
        attnT = singles.tile([128, RB * SP, 128], f32)
        nc.sync.dma_start(attnT[:], attnT_d[:])
        piT = singles.tile([128, RB * SP, 128], f32)
        nc.sync.dma_start(piT[:], piT_d[:])
        fiT = singles.tile([128, RB * SP, 128], f32)
        nc.sync.dma_start(fiT[:], fiT_d[:])
        attnT2 = singles.tile([SL2, NPAIR], f32)
        nc.sync.dma_start(attnT2[:], attnT2_d[:])
        piT2 = singles.tile([SL2, NPAIR], f32)
        nc.sync.dma_start(piT2[:], piT2_d[:])
        fiT2 = singles.tile([SL2, NPAIR], f32)
        nc.sync.dma_start(fiT2[:], fiT2_d[:])
        iotaP = singles.tile([128, 128], bf16)
        nc.sync.dma_start(iotaP[:], iotaP_d[:])
        iotaF = singles.tile([128, FD], bf16)
        nc.sync.dma_start(iotaF[:], iotaF_d[:])
        iotaF2 = singles.tile([128, 2 * FD], f16)
        nc.sync.dma_start(iotaF2[:], iotaF2_d[:])
        ident = singles.tile([128, 128], bf16)
        nc.sync.dma_start(ident[:], ident_d[:])
        whT = singles.tile([128, HB], f32)
        nc.sync.dma_start(whT[:], whT_d[:])
        wsT = singles.tile([128, HB], f32)
        nc.sync.dma_start(wsT[:], wsT_d[:])
        wxT = singles.tile([128, HB], f32)
        nc.sync.dma_start(wxT[:], wxT_d[:])
        wxb = singles.tile([128, 1], f32)
        nc.sync.dma_start(wxb[:], wxb_d[:])
        scaledT = singles.tile([128, RB * SP, 128], bf16 if scal16 else f32)
        scaledT2 = singles.tile([SL2, NPAIR], bf16 if scal16 else f32)
        pgen_all = singles.tile([128, R], f32)
        rinv_all = singles.tile([128, R], f32)
        ratio_all = singles.tile([128, R], f32)
        pgen_dram = dpool.tile([1, R], f32)
        if scal16:
            # 16-bit copies of the per-entry scalars (values are bf16/fp16
            # exact: pi<128, fi<250, fi2<500)
            piT16 = singles.tile([128, RB * SP, 128], bf16)
            nc.vector.tensor_scalar(piT16[:], piT[:], 1.0, None, Alu.mult)
            fiT16 = singles.tile([128, RB * SP, 128], bf16)
            nc.vector.tensor_scalar(fiT16[:], fiT[:], 1.0, None, Alu.mult)
            piT216 = singles.tile([SL2, NPAIR], bf16)
            nc.vector.tensor_scalar(piT216[:], piT2[:], 1.0, None, Alu.mult)
            fiT216 = singles.tile([SL2, NPAIR], f16)
            nc.vector.tensor_scalar(fiT216[:], fiT2[:], 1.0, None, Alu.mult)
            pT, fT, pT2, fT2 = piT16, fiT16, piT216, fiT216
        else:
            pT, fT, pT2, fT2 = piT, fiT, piT2, fiT2

        # --- phase 1a: p_gen per row on the PE, bounce to DRAM ---
        def _phase1a():
            xs = []
            for nm, src_d in (("c", ctxT_d), ("h", hidT_d), ("t", trgT_d)):
                xT = ph1.tile([128, HB, R], f32, tag=f"x{nm}")
                nc.sync.dma_start(
                    xT[:], src_d[:].rearrange("(c p) r -> p c r", p=128)
                )
                xs.append(xT)
            gps = pp1.tile([1, R], f32)
            for i, (xT, wT) in enumerate(zip(xs, (whT, wsT, wxT))):
                for c in range(HB):
                    nc.tensor.matmul(
                        gps[0:1, :], lhsT=wT[:, c:c + 1], rhs=xT[:, c, :],
                        start=(i == 0 and c == 0),
                        stop=(i == 2 and c == HB - 1),
                    )
            pgrow = ph1.tile([1, R], f32, tag="pgrow")
            nc.scalar.activation(
                pgrow[0:1, :], gps[0:1, :], Act.Sigmoid,
                bias=wxb[0:1, :], scale=1.0,
            )
            nc.sync.dma_start(pgen_dram[:], pgrow[0:1, :])

        # --- phase 1b: broadcast p_gen; ratio = (1-pg)/pg; scaled attn ---
        def _phase1b():
            nc.gpsimd.dma_start(
                pgen_all[:], pgen_dram[0, :].partition_broadcast(128)
            )
            nc.vector.reciprocal(rinv_all[:], pgen_all[:])
            # ratio = (1 - pg) / pg = 1/pg - 1
            nc.vector.tensor_scalar(
                ratio_all[:], rinv_all[:], 1.0, None, Alu.subtract
            )
            for blk in range(RB):
                for c in range(SP):
                    nc.vector.tensor_tensor(
                        scaledT[:, blk * SP + c, :],
                        attnT[:, blk * SP + c, :],
                        ratio_all[:, blk * 128:(blk + 1) * 128],
                        op=Alu.mult,
                    )
            # leftover entries: partitions 0:16 hold the even row (its ratio
            # in even columns), 32:48 the odd row; unused rows have attn=0 so
            # multiplying them by a garbage ratio still yields 0
            nc.vector.tensor_tensor(
                scaledT2[0:32, :], attnT2[0:32, :],
                ratio_all[0:32, 0:R:2], op=Alu.mult,
            )
            nc.vector.tensor_tensor(
                scaledT2[32:64, :], attnT2[32:64, :],
                ratio_all[32:64, 1:R:2], op=Alu.mult,
            )

        # --- phase 2: base via bf16 identity matmul + scatter matmuls ---
        def _phase2():
          for grp in range(NG):
            gr = slice(grp * G, (grp + 1) * G)
            vt = vpool.tile([128, G, FD], bf16)
            nc.sync.dma_start(vt[:], vocab_d[:, gr, :])
            ot = opool.tile([128, G, FD], bf16)
            if ablate in ("dmaonly", "aonly", "bonly"):
                nc.scalar.copy(ot[:, :, :], vt[:, :, :])
                nc.scalar.dma_start(out_d[:, gr, :], ot[:])
                if ablate == "dmaonly":
                    continue
                for j in range(0, G, 2):
                    pr = (grp * G + j) // 2
                    for jj in range(2):
                        r = grp * G + j + jj
                        blk = r // 128
                        rl = r % 128
                        for c in range(SP):
                            ch = blk * SP + c
                            if ablate == "aonly":
                                A = abpool.tile([128, 128], bf16, tag="A")
                                nc.vector.tensor_scalar(
                                    A[:], iotaP[:], pT[:, ch, rl:rl + 1],
                                    scaledT[:, ch, rl:rl + 1],
                                    Alu.is_equal, Alu.mult,
                                )
                            else:
                                Bt = abpool.tile([128, FD], bf16, tag="B")
                                nc.vector.tensor_scalar(
                                    Bt[:], iotaF[:], fT[:, ch, rl:rl + 1],
                                    None, Alu.is_equal,
                                )
                    if ablate == "aonly":
                        A2 = abpool.tile([SL2, 128], bf16, tag="A2")
                        nc.vector.tensor_scalar(
                            A2[:], iotaP[0:SL2, :], pT2[:, pr:pr + 1],
                            scaledT2[:, pr:pr + 1], Alu.is_equal, Alu.mult,
                        )
                    else:
                        B2 = abpool.tile([SL2, 2 * FD], f16, tag="B2")
                        nc.vector.tensor_scalar(
                            B2[:], iotaF2[0:SL2, :], fT2[:, pr:pr + 1], None,
                            Alu.is_equal,
                        )
                continue
            for j in range(0, G, 2):
                pr = (grp * G + j) // 2
                psb = ppool.tile([128, 2, FD], f32)
                if ablate not in ("nomm",):
                    nc.tensor.matmul(
                        psb[:, :, :], lhsT=ident[:],
                        rhs=vt[:, j:j + 2, :],
                        start=True, stop=False,
                    )
                if ablate == "novec":
                    for jj in range(2):
                        for c in range(SP):
                            nc.tensor.matmul(
                                psb[:, jj, :], lhsT=iotaP[:], rhs=iotaF[:],
                                start=False, stop=False,
                            )
                    nc.tensor.matmul(
                        psb[:, :, :], lhsT=iotaP[:],
                        rhs=iotaF2[0:128, :].bitcast(bf16),
                        start=False, stop=True,
                    )
                    for jj in range(2):
                        r = grp * G + j + jj
                        nc.scalar.mul(ot[:, j + jj, :], psb[:, jj, :],
                                      pgen_all[:, r:r + 1])
                    continue
                # Build all one-hots of the pair batched BY TYPE: alternating
                # between the two tensor_scalar flavors costs ~75ns/instr on
                # the DVE, so emit the 7 A-builds together, then the 7 B's.
                As, Bs = [], []
                for jj in range(2):
                    r = grp * G + j + jj
                    blk = r // 128
                    rl = r % 128
                    for c in range(SP):
                        ch = blk * SP + c
                        # A carries the value: A[s,p] = (pi[s]==p) * rval[s]
                        A = abpool.tile([128, 128], bf16, tag="A")
                        nc.vector.tensor_scalar(
                            A[:], iotaP[:], pT[:, ch, rl:rl + 1],
                            scaledT[:, ch, rl:rl + 1], Alu.is_equal, Alu.mult,
                        )
                        As.append(A)
                # pair-packed leftover chunk: 32 entries scatter into the
                # pair's full 500-wide PSUM block (fi' = fi + 250*parity)
                A2 = abpool.tile([SL2, 128], bf16, tag="A2")
                nc.vector.tensor_scalar(
                    A2[:], iotaP[0:SL2, :], pT2[:, pr:pr + 1],
                    scaledT2[:, pr:pr + 1], Alu.is_equal, Alu.mult,
                )
                for jj in range(2):
                    r = grp * G + j + jj
                    blk = r // 128
                    rl = r % 128
                    for c in range(SP):
                        ch = blk * SP + c
                        # B is the pure one-hot of fi (1-op, wide)
                        Bt = abpool.tile([128, FD], bf16, tag="B")
                        nc.vector.tensor_scalar(
                            Bt[:], iotaF[:], fT[:, ch, rl:rl + 1], None,
                            Alu.is_equal,
                        )
                        Bs.append(Bt)
                B2 = abpool.tile([SL2, 2 * FD], f16, tag="B2")
                nc.vector.tensor_scalar(
                    B2[:], iotaF2[0:SL2, :], fT2[:, pr:pr + 1], None,
                    Alu.is_equal,
                )
                if ablate != "nomm":
                    for jj in range(2):
                        for c in range(SP):
                            nc.tensor.matmul(
                                psb[:, jj, :], lhsT=As[jj * SP + c][:],
                                rhs=Bs[jj * SP + c][:],
                                start=False, stop=False,
                            )
                    nc.tensor.matmul(
                        psb[:, :, :], lhsT=A2[:], rhs=B2[:],
                        start=False, stop=True,
                    )
                for jj in range(2):
                    r = grp * G + j + jj
                    pg_sc = pgen_all[:, r:r + 1]
                    if ablate == "nomm":
                        nc.scalar.mul(ot[:, j + jj, :], vt[:, j + jj, :], pg_sc)
                    else:
                        # PSUM -> SBUF with the p_gen scale applied in-flight
                        nc.scalar.mul(ot[:, j + jj, :], psb[:, jj, :], pg_sc)
            nc.scalar.dma_start(out_d[:, gr, :], ot[:])

        for _ in range(rep):
            _phase1a()
            _phase1b()
            _phase2()

    nc.compile()
    _PROGRAM_CACHE[key] = nc
    return nc


def make_core_inputs(ctx, hid, trg, vocab, attn, ids, w_h, w_s, w_x_w, w_x_b,
                     R=R_FULL, FD=FD_FULL, SP=SP_FULL):
    """Host-side prep for one core: flatten rows, decompose + transpose indices.

    ctx/hid/trg: [R, H] f32; vocab: [R, 128*FD] f32; attn: [R, S'] f32;
    ids: [R, S'] int. Returns the in_map dict for this core.
    """
    RB = R // 128
    Smain = SP * 128
    f32 = np.float32

    ids = np.asarray(ids).astype(np.int64)
    pi = (ids // FD).astype(f32)
    fi = (ids % FD).astype(f32)
    attn = np.asarray(attn, dtype=f32)

    def tr(x):
        # [R, Smain] -> [RB, 128(r), SP, 128(s)] -> [s, RB, SP, r]
        t = (np.ascontiguousarray(x[:, :Smain])
             .reshape(RB, 128, SP, 128).transpose(3, 0, 2, 1))
        return np.ascontiguousarray(t.reshape(128, RB * SP, 128))

    def tr2(x, fill=0.0):
        # leftover entries of pair rows: [R, SL] -> [64, R/2] with the even
        # row's entries at partitions 0:16 and the odd row's at 32:48
        # (32-aligned partition slices on device); unused rows get `fill`
        out = np.full((64, R // 2), fill, dtype=f32)
        pair = x[:, Smain:S].reshape(R // 2, 2, SL)
        out[0:SL, :] = pair[:, 0, :].T
        out[32:32 + SL, :] = pair[:, 1, :].T
        return np.ascontiguousarray(out)

    def rep(w, n):
        return np.ascontiguousarray(
            np.broadcast_to(np.asarray(w, dtype=f32).reshape(1, n), (128, n))
        )

    fiT2 = tr2(fi, fill=-1.0)
    fiT2[32:32 + SL, :] += FD  # odd row scatters into the upper 250 columns

    # vocab: cast to bf16 and transpose to [p, r, f] so device DMAs are
    # contiguous multi-KB runs per partition (pure data movement + rounding)
    vocabT = np.ascontiguousarray(
        np.asarray(vocab, dtype=f32).astype(bfloat16)
        .reshape(R, 128, FD).transpose(1, 0, 2)
    )

    def wchunks(w):
        # [H] -> [128, H/128]: column c holds weights for h in [128c,128c+128)
        return np.ascontiguousarray(
            np.asarray(w, dtype=f32).reshape(-1).reshape(H // 128, 128).T
        )

    return {
        "ctxT": np.ascontiguousarray(np.asarray(ctx, dtype=f32).T),
        "hidT": np.ascontiguousarray(np.asarray(hid, dtype=f32).T),
        "trgT": np.ascontiguousarray(np.asarray(trg, dtype=f32).T),
        "vocabT": vocabT,
        "attnT": tr(attn),
        "piT": tr(pi),
        "fiT": tr(fi),
        "attnT2": tr2(attn),
        "piT2": tr2(pi, fill=-1.0),
        "fiT2": np.ascontiguousarray(fiT2),
        "whT": wchunks(w_h),
        "wsT": wchunks(w_s),
        "wxT": wchunks(w_x_w),
        "wxb": rep(w_x_b, 1),
        "iotaP": rep(np.arange(128, dtype=f32), 128).astype(bfloat16),
        "iotaF": rep(np.arange(FD, dtype=f32), FD).astype(bfloat16),
        "iotaF2": rep(np.arange(2 * FD, dtype=f32), 2 * FD).astype(np.float16),
        "ident": np.eye(128, dtype=np.float32).astype(bfloat16),
    }


def make_in_maps(context_vecs, hidden, trg_embs, vocab_dists, attn_dists,
                 src_ids, w_h, w_s, w_x_w, w_x_b):
    """Build the 8 per-core input dicts from full inputs."""
    context_vecs = np.asarray(context_vecs)
    hidden = np.asarray(hidden)
    trg_embs = np.asarray(trg_embs)
    vocab_dists = np.asarray(vocab_dists)
    attn_dists = np.asarray(attn_dists)
    src_ids = np.asarray(src_ids)

    in_maps = []
    for i in range(N_CORES):
        bs = slice(i * BPC, (i + 1) * BPC)
        in_maps.append(make_core_inputs(
            context_vecs[bs].reshape(R_FULL, H),
            hidden[bs].reshape(R_FULL, H),
            trg_embs[bs].reshape(R_FULL, H),
            vocab_dists[bs].reshape(R_FULL, V),
            attn_dists[bs].reshape(R_FULL, S),
            src_ids[bs].reshape(R_FULL, S),
            w_h, w_s, w_x_w, w_x_b,
        ))
    return in_maps


def kernel(context_vecs, hidden, trg_embs, vocab_dists, attn_dists,
           src_ids, pad_id, w_h, w_s, w_x_w, w_x_b):
    """Full-input entry point. Shards over 8 NeuronCores, returns [B,T,V] f32."""
    from concourse.bass_utils import run_bass_kernel_spmd

    nc = build_program()
    in_maps = make_in_maps(context_vecs, hidden, trg_embs, vocab_dists,
                           attn_dists, src_ids, w_h, w_s, w_x_w, w_x_b)
    res = run_bass_kernel_spmd(nc, in_maps, list(range(N_CORES)))
    outs = []
    for i in range(N_CORES):
        # [128, R, FD] bf16 -> [R, 128*FD] f32
        o = np.asarray(res.results[i]["outT"]).astype(np.float32)
        outs.append(o.transpose(1, 0, 2).reshape(BPC, T, V))
    return np.concatenate(outs, axis=0)


# revision 39
# speedup vs baseline: 27.8966x; 1.0425x over previous
"""CopyMechanism (pointer-generator) kernel for 8 Trainium2 NeuronCores.

Full problem: B=16, T=128, H=512, V=32000, S=400.
  gen = sigmoid(ctx@wh + hid@ws + trg@wx + b)          [B,T,1]
  out = gen * vocab_dists; out[b,t,ids[b,t,s]] += (1-gen)*attn[b,t,s]

Sharding: data-parallel over batch. Core i handles batches [2i, 2i+1]
(256 rows of T-steps). Weights replicated. No cross-core communication.

Device algorithm (per core, per row r):
  Decompose vocab index v = p*250 + f  (V = 128*250), so a row's 32000-wide
  output is an SBUF tile [128 partitions, 250 free].  Using
     out = pg * (vocab + scatter(ratio * attn)),   ratio = (1-pg)/pg,
  the whole row is accumulated in PSUM and scaled once on the way out:
   - base: one bf16 matmul per ROW PAIR with lhsT = I (identity) and
     rhs = vocab[j:j+2] starts the PSUM accumulation group with the raw
     vocab rows;
   - scatter: per row, 3 bf16 matmuls contract s-chunks of 128 (s<384):
       M[p,f] += sum_s Ap[s,p] * (onehot(fi[s])[f] * rval[s])
     where Ap = onehot(pi) is HOST-PREBUILT (pure 0/1 index data, bf16)
     and streamed in by DMA, and B = onehot(fi)*rval is built on VectorE
     by one fused tensor_scalar (is_equal, mult).  The AP-scalar fetch
     caps DVE tensor_scalar at 2x mode, so each build instruction costs
     ~0.2us regardless of op count — prebuilding A halves the DVE
     instruction count, and DMA has bandwidth to spare.  The 16 leftover
     entries (s in [384,400)) of BOTH pair rows are packed into one
     32-contraction matmul whose B' one-hot spans the pair's 500-wide
     PSUM block (fi' = fi + 250*row_parity).
   - merge: ScalarE copies PSUM->SBUF with scale = pg (per-partition AP),
     which applies pg to the base and (1-pg) to the scatter in one pass.
  p_gen is computed on the PE (12 thin matmuls against host-transposed
  activations), sigmoid on ScalarE, bounced through a DRAM scratch and
  re-loaded with a partition-broadcast AP so each row's scalar reaches all
  128 partitions.

  Memory-regime choices: vocab is shipped to the device in bf16 and in the
  [p, r, f] layout (host-side cast+transpose, pure data movement), and the
  output leaves the device in bf16 [p, r, f] (host casts back to f32 and
  untransposes).  That halves the dominant vocab+out HBM traffic and makes
  every DMA descriptor a contiguous multi-KB run per partition.  Vocab and
  one-hot loads ride the SP HWDGE ring, out stores the ACT ring.
"""

import numpy as np
from ml_dtypes import bfloat16

# ---------------------------------------------------------------------------
# problem constants (hardcoded per contract)
B, T, H, V, S = 16, 128, 512, 32000, 400
N_CORES = 8
BPC = B // N_CORES          # batches per core
R_FULL = BPC * T            # rows per core = 256
FD_FULL = V // 128          # 250
SP_FULL = 3                 # full 128-wide s-chunks (s < 384)
SL = S - 128 * SP_FULL      # leftover entries per row = 16
SL2 = 64                    # leftover pack: even row at 0:16, odd at 32:48
G_FULL = 16                 # rows per vocab DMA group

_PROGRAM_CACHE = {}


def build_program(R=R_FULL, FD=FD_FULL, SP=SP_FULL, G=G_FULL, rep=1,
                  ablate="full"):
    """Build + compile the per-core Bass program. Same program for all cores.

    rep : repeat the whole body rep times (identical output; used for
          differential device-time measurement).
    ablate: "full" | "dmaonly" | "nomm" (builds, no matmuls) |
            "novec" (matmuls on constants, no builds)
    """
    key = (R, FD, SP, G, rep, ablate)
    if key in _PROGRAM_CACHE:
        return _PROGRAM_CACHE[key]

    from contextlib import ExitStack

    import concourse.bass as bass
    import concourse.tile as tile
    from concourse import bacc, mybir

    f32 = mybir.dt.float32
    bf16 = mybir.dt.bfloat16
    f16 = mybir.dt.float16
    Alu = mybir.AluOpType
    Act = mybir.ActivationFunctionType
    RB = R // 128
    NG = R // G
    NPAIR = R // 2
    HB = H // 128
    PPG = G // 2  # pairs per group
    assert R % 128 == 0 and R % G == 0 and G % 2 == 0

    nc = bacc.Bacc("TRN2", target_bir_lowering=False, debug=False)

    # host-transposed activations [H, R] for the PE-side p_gen dot products
    ctxT_d = nc.dram_tensor("ctxT", [H, R], f32, kind="ExternalInput")
    hidT_d = nc.dram_tensor("hidT", [H, R], f32, kind="ExternalInput")
    trgT_d = nc.dram_tensor("trgT", [H, R], f32, kind="ExternalInput")
    vocab_d = nc.dram_tensor("vocabT", [128, R, FD], bf16, kind="ExternalInput")
    # host-prebuilt pure one-hots of pi: Ap[s, r*SP+c, p] (bf16 0/1)
    Ap_d = nc.dram_tensor("Ap", [128, R * SP, 128], bf16, kind="ExternalInput")
    A2p_d = nc.dram_tensor("A2p", [SL2, NPAIR, 128], bf16, kind="ExternalInput")
    attnT_d = nc.dram_tensor("attnT", [128, RB * SP, 128], f32, kind="ExternalInput")
    fiT_d = nc.dram_tensor("fiT", [128, RB * SP, 128], f32, kind="ExternalInput")
    attnT2_d = nc.dram_tensor("attnT2", [SL2, NPAIR], f32, kind="ExternalInput")
    fiT2_d = nc.dram_tensor("fiT2", [SL2, NPAIR], f32, kind="ExternalInput")
    # weights in [hl, c] chunk layout for the PE dot products
    whT_d = nc.dram_tensor("whT", [128, HB], f32, kind="ExternalInput")
    wsT_d = nc.dram_tensor("wsT", [128, HB], f32, kind="ExternalInput")
    wxT_d = nc.dram_tensor("wxT", [128, HB], f32, kind="ExternalInput")
    wxb_d = nc.dram_tensor("wxb", [128, 1], f32, kind="ExternalInput")
    iotaF_d = nc.dram_tensor("iotaF", [128, FD], bf16, kind="ExternalInput")
    # fp16 (not bf16): bf16 can't represent odd integers above 256 exactly
    iotaF2_d = nc.dram_tensor("iotaF2", [128, 2 * FD], f16, kind="ExternalInput")
    ident_d = nc.dram_tensor("ident", [128, 128], bf16, kind="ExternalInput")
    out_d = nc.dram_tensor("outT", [128, R, FD], bf16, kind="ExternalOutput")

    with tile.TileContext(nc) as tc, ExitStack() as es:
        singles = es.enter_context(tc.tile_pool(name="singles", bufs=1))
        ph1 = es.enter_context(tc.tile_pool(name="ph1", bufs=2))
        vpool = es.enter_context(tc.tile_pool(name="vpool", bufs=4))
        opool = es.enter_context(tc.tile_pool(name="opool", bufs=4))
        apool = es.enter_context(tc.tile_pool(name="ap", bufs=3))
        bpool = es.enter_context(tc.tile_pool(name="b", bufs=16))
        ppool = es.enter_context(tc.tile_pool(name="psum", bufs=7, space="PSUM"))
        pp1 = es.enter_context(tc.tile_pool(name="psum1", bufs=1, space="PSUM"))
        dpool = es.enter_context(tc.tile_pool(name="dram", bufs=1, space="DRAM"))

        # --- constants / small inputs ---
        attnT = singles.tile([128, RB * SP, 128], f32)
        nc.sync.dma_start(attnT[:], attnT_d[:])
        fiT = singles.tile([128, RB * SP, 128], f32)
        nc.sync.dma_start(fiT[:], fiT_d[:])
        attnT2 = singles.tile([SL2, NPAIR], f32)
        nc.sync.dma_start(attnT2[:], attnT2_d[:])
        fiT2 = singles.tile([SL2, NPAIR], f32)
        nc.sync.dma_start(fiT2[:], fiT2_d[:])
        iotaF = singles.tile([128, FD], bf16)
        nc.sync.dma_start(iotaF[:], iotaF_d[:])
        iotaF2 = singles.tile([128, 2 * FD], f16)
        nc.sync.dma_start(iotaF2[:], iotaF2_d[:])
        ident = singles.tile([128, 128], bf16)
        nc.sync.dma_start(ident[:], ident_d[:])
        whT = singles.tile([128, HB], f32)
        nc.sync.dma_start(whT[:], whT_d[:])
        wsT = singles.tile([128, HB], f32)
        nc.sync.dma_start(wsT[:], wsT_d[:])
        wxT = singles.tile([128, HB], f32)
        nc.sync.dma_start(wxT[:], wxT_d[:])
        wxb = singles.tile([128, 1], f32)
        nc.sync.dma_start(wxb[:], wxb_d[:])
        scaledT = singles.tile([128, RB * SP, 128], f32)
        scaledT2 = singles.tile([SL2, NPAIR], f32)
        pgen_all = singles.tile([128, R], f32)
        rinv_all = singles.tile([128, R], f32)
        ratio_all = singles.tile([128, R], f32)
        pgen_dram = dpool.tile([1, R], f32)

        # --- phase 1a: p_gen per row on the PE, bounce to DRAM ---
        def _phase1a():
            xs = []
            for nm, src_d in (("c", ctxT_d), ("h", hidT_d), ("t", trgT_d)):
                xT = ph1.tile([128, HB, R], f32, tag=f"x{nm}")
                nc.sync.dma_start(
                    xT[:], src_d[:].rearrange("(c p) r -> p c r", p=128)
                )
                xs.append(xT)
            gps = pp1.tile([1, R], f32)
            for i, (xT, wT) in enumerate(zip(xs, (whT, wsT, wxT))):
                for c in range(HB):
                    nc.tensor.matmul(
                        gps[0:1, :], lhsT=wT[:, c:c + 1], rhs=xT[:, c, :],
                        start=(i == 0 and c == 0),
                        stop=(i == 2 and c == HB - 1),
                    )
            pgrow = ph1.tile([1, R], f32, tag="pgrow")
            nc.scalar.activation(
                pgrow[0:1, :], gps[0:1, :], Act.Sigmoid,
                bias=wxb[0:1, :], scale=1.0,
            )
            nc.sync.dma_start(pgen_dram[:], pgrow[0:1, :])

        # --- phase 1b: broadcast p_gen; ratio = (1-pg)/pg; scaled attn ---
        def _phase1b():
            nc.gpsimd.dma_start(
                pgen_all[:], pgen_dram[0, :].partition_broadcast(128)
            )
            nc.vector.reciprocal(rinv_all[:], pgen_all[:])
            # ratio = (1 - pg) / pg = 1/pg - 1
            nc.vector.tensor_scalar(
                ratio_all[:], rinv_all[:], 1.0, None, Alu.subtract
            )
            for blk in range(RB):
                for c in range(SP):
                    nc.vector.tensor_tensor(
                        scaledT[:, blk * SP + c, :],
                        attnT[:, blk * SP + c, :],
                        ratio_all[:, blk * 128:(blk + 1) * 128],
                        op=Alu.mult,
                    )
            # leftover entries: partitions 0:16 hold the even row (its ratio
            # in even columns), 32:48 the odd row; unused rows have attn=0 so
            # multiplying them by a garbage ratio still yields 0
            nc.vector.tensor_tensor(
                scaledT2[0:32, :], attnT2[0:32, :],
                ratio_all[0:32, 0:R:2], op=Alu.mult,
            )
            nc.vector.tensor_tensor(
                scaledT2[32:64, :], attnT2[32:64, :],
                ratio_all[32:64, 1:R:2], op=Alu.mult,
            )

        # --- phase 2: base via bf16 identity matmul + scatter matmuls ---
        def _phase2():
          for grp in range(NG):
            gr = slice(grp * G, (grp + 1) * G)
            vt = vpool.tile([128, G, FD], bf16)
            nc.sync.dma_start(vt[:], vocab_d[:, gr, :])
            ot = opool.tile([128, G, FD], bf16)
            if ablate == "dmaonly":
                nc.scalar.copy(ot[:, :, :], vt[:, :, :])
                nc.scalar.dma_start(out_d[:, gr, :], ot[:])
                continue
            # group's prebuilt one-hots (pure DMA traffic)
            apt = apool.tile([128, G * SP, 128], bf16, tag="ap")
            nc.sync.dma_start(
                apt[:], Ap_d[:, grp * G * SP:(grp + 1) * G * SP, :]
            )
            a2t = apool.tile([SL2, PPG, 128], bf16, tag="a2p")
            nc.sync.dma_start(
                a2t[:], A2p_d[:, grp * PPG:(grp + 1) * PPG, :]
            )
            for j in range(0, G, 2):
                pr = (grp * G + j) // 2
                psb = ppool.tile([128, 2, FD], f32)
                if ablate != "nomm":
                    nc.tensor.matmul(
                        psb[:, :, :], lhsT=ident[:],
                        rhs=vt[:, j:j + 2, :],
                        start=True, stop=False,
                    )
                if ablate == "novec":
                    for jj in range(2):
                        for c in range(SP):
                            nc.tensor.matmul(
                                psb[:, jj, :],
                                lhsT=apt[:, (j + jj) * SP + c, :],
                                rhs=iotaF[:],
                                start=False, stop=False,
                            )
                    nc.tensor.matmul(
                        psb[:, :, :], lhsT=a2t[:, j // 2, :],
                        rhs=iotaF2[0:SL2, :].bitcast(bf16),
                        start=False, stop=True,
                    )
                    for jj in range(2):
                        r = grp * G + j + jj
                        nc.scalar.mul(ot[:, j + jj, :], psb[:, jj, :],
                                      pgen_all[:, r:r + 1])
                    continue
                # B carries the value: B[s,f] = (fi[s]==f) * rval[s]
                Bs = []
                for jj in range(2):
                    r = grp * G + j + jj
                    blk = r // 128
                    rl = r % 128
                    for c in range(SP):
                        ch = blk * SP + c
                        Bt = bpool.tile([128, FD], bf16, tag="B")
                        nc.vector.tensor_scalar(
                            Bt[:], iotaF[:], fiT[:, ch, rl:rl + 1],
                            scaledT[:, ch, rl:rl + 1], Alu.is_equal, Alu.mult,
                        )
                        Bs.append(Bt)
                # pair-packed leftover chunk: 32 entries scatter into the
                # pair's full 500-wide PSUM block (fi' = fi + 250*parity)
                B2 = bpool.tile([SL2, 2 * FD], f16, tag="B2")
                nc.vector.tensor_scalar(
                    B2[:], iotaF2[0:SL2, :], fiT2[:, pr:pr + 1],
                    scaledT2[:, pr:pr + 1], Alu.is_equal, Alu.mult,
                )
                if ablate != "nomm":
                    for jj in range(2):
                        for c in range(SP):
                            nc.tensor.matmul(
                                psb[:, jj, :],
                                lhsT=apt[:, (j + jj) * SP + c, :],
                                rhs=Bs[jj * SP + c][:],
                                start=False, stop=False,
                            )
                    nc.tensor.matmul(
                        psb[:, :, :], lhsT=a2t[:, j // 2, :], rhs=B2[:],
                        start=False, stop=True,
                    )
                for jj in range(2):
                    r = grp * G + j + jj
                    pg_sc = pgen_all[:, r:r + 1]
                    if ablate == "nomm":
                        nc.scalar.mul(ot[:, j + jj, :], vt[:, j + jj, :], pg_sc)
                    else:
                        # PSUM -> SBUF with the p_gen scale applied in-flight
                        nc.scalar.mul(ot[:, j + jj, :], psb[:, jj, :], pg_sc)
            nc.scalar.dma_start(out_d[:, gr, :], ot[:])

        for _ in range(rep):
            _phase1a()
            _phase1b()
            _phase2()

    nc.compile()
    _PROGRAM_CACHE[key] = nc
    return nc


def make_core_inputs(ctx, hid, trg, vocab, attn, ids, w_h, w_s, w_x_w, w_x_b,
                     R=R_FULL, FD=FD_FULL, SP=SP_FULL):
    """Host-side prep for one core: flatten rows, decompose + transpose
    indices, prebuild the pure pi one-hots (integer-only preprocessing)."""
    RB = R // 128
    Smain = SP * 128
    f32 = np.float32

    ids = np.asarray(ids).astype(np.int64)
    pi = (ids // FD).astype(f32)
    fi = (ids % FD).astype(f32)
    attn = np.asarray(attn, dtype=f32)

    def tr(x):
        # [R, Smain] -> [RB, 128(r), SP, 128(s)] -> [s, RB, SP, r]
        t = (np.ascontiguousarray(x[:, :Smain])
             .reshape(RB, 128, SP, 128).transpose(3, 0, 2, 1))
        return np.ascontiguousarray(t.reshape(128, RB * SP, 128))

    def tr2(x, fill=0.0):
        # leftover entries of pair rows: [R, SL] -> [64, R/2] with the even
        # row's entries at partitions 0:16 and the odd row's at 32:48
        # (32-aligned partition slices on device); unused rows get `fill`
        out = np.full((64, R // 2), fill, dtype=f32)
        pair = x[:, Smain:S].reshape(R // 2, 2, SL)
        out[0:SL, :] = pair[:, 0, :].T
        out[32:32 + SL, :] = pair[:, 1, :].T
        return np.ascontiguousarray(out)

    def rep(w, n):
        return np.ascontiguousarray(
            np.broadcast_to(np.asarray(w, dtype=f32).reshape(1, n), (128, n))
        )

    fiT2 = tr2(fi, fill=-1.0)
    fiT2[32:32 + SL, :] += FD  # odd row scatters into the upper 250 columns

    # prebuilt pure one-hots of pi (0/1 in bf16 — exact):
    # Ap[s, r*SP+c, p] = (pi[r, 128c+s] == p)
    pi_i = (ids[:, :Smain] // FD).astype(np.int16).reshape(R, SP, 128)
    Ap = (pi_i.transpose(2, 0, 1)[..., None]
          == np.arange(128, dtype=np.int16)).astype(bfloat16)
    Ap = np.ascontiguousarray(Ap.reshape(128, R * SP, 128))
    # A2p[k, pair, p] = (piT2[k, pair] == p); fill rows (-1) give all-zero
    piT2 = tr2(pi, fill=-1.0)
    A2p = np.ascontiguousarray(
        (piT2[..., None] == np.arange(128, dtype=f32)).astype(bfloat16)
    )

    # vocab: cast to bf16 and transpose to [p, r, f] so device DMAs are
    # contiguous multi-KB runs per partition (pure data movement + rounding)
    vocabT = np.ascontiguousarray(
        np.asarray(vocab, dtype=f32).astype(bfloat16)
        .reshape(R, 128, FD).transpose(1, 0, 2)
    )

    def wchunks(w):
        # [H] -> [128, H/128]: column c holds weights for h in [128c,128c+128)
        return np.ascontiguousarray(
            np.asarray(w, dtype=f32).reshape(-1).reshape(H // 128, 128).T
        )

    return {
        "ctxT": np.ascontiguousarray(np.asarray(ctx, dtype=f32).T),
        "hidT": np.ascontiguousarray(np.asarray(hid, dtype=f32).T),
        "trgT": np.ascontiguousarray(np.asarray(trg, dtype=f32).T),
        "vocabT": vocabT,
        "Ap": Ap,
        "A2p": A2p,
        "attnT": tr(attn),
        "fiT": tr(fi),
        "attnT2": tr2(attn),
        "fiT2": np.ascontiguousarray(fiT2),
        "whT": wchunks(w_h),
        "wsT": wchunks(w_s),
        "wxT": wchunks(w_x_w),
        "wxb": rep(w_x_b, 1),
        "iotaF": rep(np.arange(FD, dtype=f32), FD).astype(bfloat16),
        "iotaF2": rep(np.arange(2 * FD, dtype=f32), 2 * FD).astype(np.float16),
        "ident": np.eye(128, dtype=np.float32).astype(bfloat16),
    }


def make_in_maps(context_vecs, hidden, trg_embs, vocab_dists, attn_dists,
                 src_ids, w_h, w_s, w_x_w, w_x_b):
    """Build the 8 per-core input dicts from full inputs."""
    context_vecs = np.asarray(context_vecs)
    hidden = np.asarray(hidden)
    trg_embs = np.asarray(trg_embs)
    vocab_dists = np.asarray(vocab_dists)
    attn_dists = np.asarray(attn_dists)
    src_ids = np.asarray(src_ids)

    in_maps = []
    for i in range(N_CORES):
        bs = slice(i * BPC, (i + 1) * BPC)
        in_maps.append(make_core_inputs(
            context_vecs[bs].reshape(R_FULL, H),
            hidden[bs].reshape(R_FULL, H),
            trg_embs[bs].reshape(R_FULL, H),
            vocab_dists[bs].reshape(R_FULL, V),
            attn_dists[bs].reshape(R_FULL, S),
            src_ids[bs].reshape(R_FULL, S),
            w_h, w_s, w_x_w, w_x_b,
        ))
    return in_maps


def kernel(context_vecs, hidden, trg_embs, vocab_dists, attn_dists,
           src_ids, pad_id, w_h, w_s, w_x_w, w_x_b):
    """Full-input entry point. Shards over 8 NeuronCores, returns [B,T,V] f32."""
    from concourse.bass_utils import run_bass_kernel_spmd

    nc = build_program()
    in_maps = make_in_maps(context_vecs, hidden, trg_embs, vocab_dists,
                           attn_dists, src_ids, w_h, w_s, w_x_w, w_x_b)
    res = run_bass_kernel_spmd(nc, in_maps, list(range(N_CORES)))
    outs = []
    for i in range(N_CORES):
        # [128, R, FD] bf16 -> [R, 128*FD] f32
        o = np.asarray(res.results[i]["outT"]).astype(np.float32)
        outs.append(o.transpose(1, 0, 2).reshape(BPC, T, V))
    return np.concatenate(outs, axis=0)


# revision 40
# speedup vs baseline: 44.6683x; 1.6012x over previous
"""CopyMechanism (pointer-generator) kernel for 8 Trainium2 NeuronCores.

Full problem: B=16, T=128, H=512, V=32000, S=400.
  gen = sigmoid(ctx@wh + hid@ws + trg@wx + b)          [B,T,1]
  out = gen * vocab_dists; out[b,t,ids[b,t,s]] += (1-gen)*attn[b,t,s]

Sharding: data-parallel over batch. Core i handles batches [2i, 2i+1]
(256 rows of T-steps). Weights replicated. No cross-core communication.

Device algorithm (per core, per row r):
  Decompose vocab index v = p*250 + f  (V = 128*250), so a row's 32000-wide
  output is an SBUF tile [128 partitions, 250 free].  Using
     out = pg * (vocab + scatter(ratio * attn)),   ratio = (1-pg)/pg,
  the whole row is accumulated in PSUM and scaled once on the way out:
   - base: one bf16 matmul per ROW PAIR with lhsT = I (identity) and
     rhs = vocab[j:j+2] starts the PSUM accumulation group with the raw
     vocab rows;
   - scatter: per row, 3 bf16 matmuls contract s-chunks of 128 (s<384):
       M[p,f] += sum_s Ap[s,p] * (onehot(fi[s])[f] * rval[s])
     where Ap = onehot(pi) is HOST-PREBUILT (pure 0/1 index data, bf16)
     and streamed in by DMA, and B = onehot(fi)*rval is built on VectorE
     by one fused tensor_scalar (is_equal, mult).  The AP-scalar fetch
     caps DVE tensor_scalar at 2x mode, so each build instruction costs
     ~0.2us regardless of op count — prebuilding A halves the DVE
     instruction count, and DMA has bandwidth to spare.  The 16 leftover
     entries (s in [384,400)) of BOTH pair rows are packed into one
     32-contraction matmul whose B' one-hot spans the pair's 500-wide
     PSUM block (fi' = fi + 250*row_parity).
   - merge: ScalarE copies PSUM->SBUF with scale = pg (per-partition AP),
     which applies pg to the base and (1-pg) to the scatter in one pass.
  p_gen is computed on the PE (12 thin matmuls against host-transposed
  activations), sigmoid on ScalarE, bounced through a DRAM scratch and
  re-loaded with a partition-broadcast AP so each row's scalar reaches all
  128 partitions.

  Memory-regime choices: vocab is shipped to the device in bf16 and in the
  [p, r, f] layout (host-side cast+transpose, pure data movement), and the
  output leaves the device in bf16 [p, r, f] (host casts back to f32 and
  untransposes).  That halves the dominant vocab+out HBM traffic and makes
  every DMA descriptor a contiguous multi-KB run per partition.  Vocab and
  one-hot loads ride the SP HWDGE ring, out stores the ACT ring.
"""

import numpy as np
from ml_dtypes import bfloat16

# ---------------------------------------------------------------------------
# problem constants (hardcoded per contract)
B, T, H, V, S = 16, 128, 512, 32000, 400
N_CORES = 8
BPC = B // N_CORES          # batches per core
R_FULL = BPC * T            # rows per core = 256
FD_FULL = V // 128          # 250
SP_FULL = 3                 # full 128-wide s-chunks (s < 384)
SL = S - 128 * SP_FULL      # leftover entries per row = 16
SL2 = 64                    # leftover pack: even row at 0:16, odd at 32:48
G_FULL = 16                 # rows per vocab DMA group

_PROGRAM_CACHE = {}


def build_program(R=R_FULL, FD=FD_FULL, SP=SP_FULL, G=G_FULL, rep=1,
                  ablate="full"):
    """Build + compile the per-core Bass program. Same program for all cores.

    rep : repeat the whole body rep times (identical output; used for
          differential device-time measurement).
    ablate: "full" | "dmaonly" | "nomm" (builds, no matmuls) |
            "novec" (matmuls on constants, no builds)
    """
    key = (R, FD, SP, G, rep, ablate)
    if key in _PROGRAM_CACHE:
        return _PROGRAM_CACHE[key]

    from contextlib import ExitStack

    import concourse.bass as bass
    import concourse.tile as tile
    from concourse import bacc, mybir

    f32 = mybir.dt.float32
    bf16 = mybir.dt.bfloat16
    f16 = mybir.dt.float16
    Alu = mybir.AluOpType
    Act = mybir.ActivationFunctionType
    RB = R // 128
    NG = R // G
    NPAIR = R // 2
    HB = H // 128
    PPG = G // 2  # pairs per group
    assert R % 128 == 0 and R % G == 0 and G % 2 == 0

    nc = bacc.Bacc("TRN2", target_bir_lowering=False, debug=False)

    # host-transposed activations [H, R] for the PE-side p_gen dot products
    ctxT_d = nc.dram_tensor("ctxT", [H, R], f32, kind="ExternalInput")
    hidT_d = nc.dram_tensor("hidT", [H, R], f32, kind="ExternalInput")
    trgT_d = nc.dram_tensor("trgT", [H, R], f32, kind="ExternalInput")
    vocab_d = nc.dram_tensor("vocabT", [128, R, FD], bf16, kind="ExternalInput")
    # host-prebuilt pure one-hots of pi: Ap[s, r*SP+c, p] (bf16 0/1)
    Ap_d = nc.dram_tensor("Ap", [128, R * SP, 128], bf16, kind="ExternalInput")
    A2p_d = nc.dram_tensor("A2p", [SL2, NPAIR, 128], bf16, kind="ExternalInput")
    attnT_d = nc.dram_tensor("attnT", [128, RB * SP, 128], f32, kind="ExternalInput")
    fiT_d = nc.dram_tensor("fiT", [128, RB * SP, 128], f32, kind="ExternalInput")
    attnT2_d = nc.dram_tensor("attnT2", [SL2, NPAIR], f32, kind="ExternalInput")
    fiT2_d = nc.dram_tensor("fiT2", [SL2, NPAIR], f32, kind="ExternalInput")
    # weights in [hl, c] chunk layout for the PE dot products
    whT_d = nc.dram_tensor("whT", [128, HB], f32, kind="ExternalInput")
    wsT_d = nc.dram_tensor("wsT", [128, HB], f32, kind="ExternalInput")
    wxT_d = nc.dram_tensor("wxT", [128, HB], f32, kind="ExternalInput")
    wxb_d = nc.dram_tensor("wxb", [128, 1], f32, kind="ExternalInput")
    iotaF_d = nc.dram_tensor("iotaF", [128, FD], bf16, kind="ExternalInput")
    # fp16 (not bf16): bf16 can't represent odd integers above 256 exactly
    iotaF2_d = nc.dram_tensor("iotaF2", [128, 2 * FD], f16, kind="ExternalInput")
    ident_d = nc.dram_tensor("ident", [128, 128], bf16, kind="ExternalInput")
    out_d = nc.dram_tensor("outT", [128, R, FD], bf16, kind="ExternalOutput")

    with tile.TileContext(nc) as tc, ExitStack() as es:
        singles = es.enter_context(tc.tile_pool(name="singles", bufs=1))
        ph1 = es.enter_context(tc.tile_pool(name="ph1", bufs=2))
        vpool = es.enter_context(tc.tile_pool(name="vpool", bufs=5))
        opool = es.enter_context(tc.tile_pool(name="opool", bufs=5))
        apool = es.enter_context(tc.tile_pool(name="ap", bufs=3))
        bpool = es.enter_context(tc.tile_pool(name="b", bufs=16))
        ppool = es.enter_context(tc.tile_pool(name="psum", bufs=7, space="PSUM"))
        pp1 = es.enter_context(tc.tile_pool(name="psum1", bufs=1, space="PSUM"))
        dpool = es.enter_context(tc.tile_pool(name="dram", bufs=1, space="DRAM"))

        # --- constants / small inputs ---
        attnT = singles.tile([128, RB * SP, 128], f32)
        nc.sync.dma_start(attnT[:], attnT_d[:])
        fiT = singles.tile([128, RB * SP, 128], f32)
        nc.sync.dma_start(fiT[:], fiT_d[:])
        attnT2 = singles.tile([SL2, NPAIR], f32)
        nc.sync.dma_start(attnT2[:], attnT2_d[:])
        fiT2 = singles.tile([SL2, NPAIR], f32)
        nc.sync.dma_start(fiT2[:], fiT2_d[:])
        iotaF = singles.tile([128, FD], bf16)
        nc.sync.dma_start(iotaF[:], iotaF_d[:])
        iotaF2 = singles.tile([128, 2 * FD], f16)
        nc.sync.dma_start(iotaF2[:], iotaF2_d[:])
        ident = singles.tile([128, 128], bf16)
        nc.sync.dma_start(ident[:], ident_d[:])
        whT = singles.tile([128, HB], f32)
        nc.sync.dma_start(whT[:], whT_d[:])
        wsT = singles.tile([128, HB], f32)
        nc.sync.dma_start(wsT[:], wsT_d[:])
        wxT = singles.tile([128, HB], f32)
        nc.sync.dma_start(wxT[:], wxT_d[:])
        wxb = singles.tile([128, 1], f32)
        nc.sync.dma_start(wxb[:], wxb_d[:])
        scaledT = singles.tile([128, RB * SP, 128], f32)
        scaledT2 = singles.tile([SL2, NPAIR], f32)
        pgen_all = singles.tile([128, R], f32)
        rinv_all = singles.tile([128, R], f32)
        ratio_all = singles.tile([128, R], f32)
        pgen_dram = dpool.tile([1, R], f32)

        # --- phase 1a: p_gen per row on the PE, bounce to DRAM ---
        def _phase1a():
            xs = []
            for nm, src_d in (("c", ctxT_d), ("h", hidT_d), ("t", trgT_d)):
                xT = ph1.tile([128, HB, R], f32, tag=f"x{nm}")
                nc.sync.dma_start(
                    xT[:], src_d[:].rearrange("(c p) r -> p c r", p=128)
                )
                xs.append(xT)
            gps = pp1.tile([1, R], f32)
            for i, (xT, wT) in enumerate(zip(xs, (whT, wsT, wxT))):
                for c in range(HB):
                    nc.tensor.matmul(
                        gps[0:1, :], lhsT=wT[:, c:c + 1], rhs=xT[:, c, :],
                        start=(i == 0 and c == 0),
                        stop=(i == 2 and c == HB - 1),
                    )
            pgrow = ph1.tile([1, R], f32, tag="pgrow")
            nc.scalar.activation(
                pgrow[0:1, :], gps[0:1, :], Act.Sigmoid,
                bias=wxb[0:1, :], scale=1.0,
            )
            nc.sync.dma_start(pgen_dram[:], pgrow[0:1, :])

        # --- phase 1b: broadcast p_gen; ratio = (1-pg)/pg; scaled attn ---
        def _phase1b():
            nc.gpsimd.dma_start(
                pgen_all[:], pgen_dram[0, :].partition_broadcast(128)
            )
            nc.vector.reciprocal(rinv_all[:], pgen_all[:])
            # ratio = (1 - pg) / pg = 1/pg - 1
            nc.vector.tensor_scalar(
                ratio_all[:], rinv_all[:], 1.0, None, Alu.subtract
            )
            for blk in range(RB):
                for c in range(SP):
                    nc.vector.tensor_tensor(
                        scaledT[:, blk * SP + c, :],
                        attnT[:, blk * SP + c, :],
                        ratio_all[:, blk * 128:(blk + 1) * 128],
                        op=Alu.mult,
                    )
            # leftover entries: partitions 0:16 hold the even row (its ratio
            # in even columns), 32:48 the odd row; unused rows have attn=0 so
            # multiplying them by a garbage ratio still yields 0
            nc.vector.tensor_tensor(
                scaledT2[0:32, :], attnT2[0:32, :],
                ratio_all[0:32, 0:R:2], op=Alu.mult,
            )
            nc.vector.tensor_tensor(
                scaledT2[32:64, :], attnT2[32:64, :],
                ratio_all[32:64, 1:R:2], op=Alu.mult,
            )

        # --- phase 2: base via bf16 identity matmul + scatter matmuls ---
        def _phase2():
          for grp in range(NG):
            gr = slice(grp * G, (grp + 1) * G)
            vt = vpool.tile([128, G, FD], bf16)
            nc.sync.dma_start(vt[:], vocab_d[:, gr, :])
            ot = opool.tile([128, G, FD], bf16)
            if ablate == "dmaonly":
                nc.scalar.copy(ot[:, :, :], vt[:, :, :])
                nc.scalar.dma_start(out_d[:, gr, :], ot[:])
                continue
            # group's prebuilt one-hots (pure DMA traffic)
            apt = apool.tile([128, G * SP, 128], bf16, tag="ap")
            nc.sync.dma_start(
                apt[:], Ap_d[:, grp * G * SP:(grp + 1) * G * SP, :]
            )
            a2t = apool.tile([SL2, PPG, 128], bf16, tag="a2p")
            nc.sync.dma_start(
                a2t[:], A2p_d[:, grp * PPG:(grp + 1) * PPG, :]
            )
            for j in range(0, G, 2):
                pr = (grp * G + j) // 2
                psb = ppool.tile([128, 2, FD], f32)
                if ablate != "nomm":
                    nc.tensor.matmul(
                        psb[:, :, :], lhsT=ident[:],
                        rhs=vt[:, j:j + 2, :],
                        start=True, stop=False,
                    )
                if ablate == "novec":
                    for jj in range(2):
                        for c in range(SP):
                            nc.tensor.matmul(
                                psb[:, jj, :],
                                lhsT=apt[:, (j + jj) * SP + c, :],
                                rhs=iotaF[:],
                                start=False, stop=False,
                            )
                    nc.tensor.matmul(
                        psb[:, :, :], lhsT=a2t[:, j // 2, :],
                        rhs=iotaF2[0:SL2, :].bitcast(bf16),
                        start=False, stop=True,
                    )
                    for jj in range(2):
                        r = grp * G + j + jj
                        nc.scalar.mul(ot[:, j + jj, :], psb[:, jj, :],
                                      pgen_all[:, r:r + 1])
                    continue
                # B carries the value: B[s,f] = (fi[s]==f) * rval[s]
                Bs = []
                for jj in range(2):
                    r = grp * G + j + jj
                    blk = r // 128
                    rl = r % 128
                    for c in range(SP):
                        ch = blk * SP + c
                        Bt = bpool.tile([128, FD], bf16, tag="B")
                        nc.vector.tensor_scalar(
                            Bt[:], iotaF[:], fiT[:, ch, rl:rl + 1],
                            scaledT[:, ch, rl:rl + 1], Alu.is_equal, Alu.mult,
                        )
                        Bs.append(Bt)
                # pair-packed leftover chunk: 32 entries scatter into the
                # pair's full 500-wide PSUM block (fi' = fi + 250*parity)
                B2 = bpool.tile([SL2, 2 * FD], f16, tag="B2")
                nc.vector.tensor_scalar(
                    B2[:], iotaF2[0:SL2, :], fiT2[:, pr:pr + 1],
                    scaledT2[:, pr:pr + 1], Alu.is_equal, Alu.mult,
                )
                if ablate != "nomm":
                    for jj in range(2):
                        for c in range(SP):
                            nc.tensor.matmul(
                                psb[:, jj, :],
                                lhsT=apt[:, (j + jj) * SP + c, :],
                                rhs=Bs[jj * SP + c][:],
                                start=False, stop=False,
                            )
                    nc.tensor.matmul(
                        psb[:, :, :], lhsT=a2t[:, j // 2, :], rhs=B2[:],
                        start=False, stop=True,
                    )
                for jj in range(2):
                    r = grp * G + j + jj
                    pg_sc = pgen_all[:, r:r + 1]
                    if ablate == "nomm":
                        nc.scalar.mul(ot[:, j + jj, :], vt[:, j + jj, :], pg_sc)
                    else:
                        # PSUM -> SBUF with the p_gen scale applied in-flight
                        nc.scalar.mul(ot[:, j + jj, :], psb[:, jj, :], pg_sc)
            nc.scalar.dma_start(out_d[:, gr, :], ot[:])

        for _ in range(rep):
            _phase1a()
            _phase1b()
            _phase2()

    nc.compile()
    _PROGRAM_CACHE[key] = nc
    return nc


def make_core_inputs(ctx, hid, trg, vocab, attn, ids, w_h, w_s, w_x_w, w_x_b,
                     R=R_FULL, FD=FD_FULL, SP=SP_FULL):
    """Host-side prep for one core: flatten rows, decompose + transpose
    indices, prebuild the pure pi one-hots (integer-only preprocessing)."""
    RB = R // 128
    Smain = SP * 128
    f32 = np.float32

    ids = np.asarray(ids).astype(np.int64)
    pi = (ids // FD).astype(f32)
    fi = (ids % FD).astype(f32)
    attn = np.asarray(attn, dtype=f32)

    def tr(x):
        # [R, Smain] -> [RB, 128(r), SP, 128(s)] -> [s, RB, SP, r]
        t = (np.ascontiguousarray(x[:, :Smain])
             .reshape(RB, 128, SP, 128).transpose(3, 0, 2, 1))
        return np.ascontiguousarray(t.reshape(128, RB * SP, 128))

    def tr2(x, fill=0.0):
        # leftover entries of pair rows: [R, SL] -> [64, R/2] with the even
        # row's entries at partitions 0:16 and the odd row's at 32:48
        # (32-aligned partition slices on device); unused rows get `fill`
        out = np.full((64, R // 2), fill, dtype=f32)
        pair = x[:, Smain:S].reshape(R // 2, 2, SL)
        out[0:SL, :] = pair[:, 0, :].T
        out[32:32 + SL, :] = pair[:, 1, :].T
        return np.ascontiguousarray(out)

    def rep(w, n):
        return np.ascontiguousarray(
            np.broadcast_to(np.asarray(w, dtype=f32).reshape(1, n), (128, n))
        )

    fiT2 = tr2(fi, fill=-1.0)
    fiT2[32:32 + SL, :] += FD  # odd row scatters into the upper 250 columns

    # prebuilt pure one-hots of pi (0/1 in bf16 — exact):
    # Ap[s, r*SP+c, p] = (pi[r, 128c+s] == p)
    pi_i = (ids[:, :Smain] // FD).astype(np.int16).reshape(R, SP, 128)
    Ap = (pi_i.transpose(2, 0, 1)[..., None]
          == np.arange(128, dtype=np.int16)).astype(bfloat16)
    Ap = np.ascontiguousarray(Ap.reshape(128, R * SP, 128))
    # A2p[k, pair, p] = (piT2[k, pair] == p); fill rows (-1) give all-zero
    piT2 = tr2(pi, fill=-1.0)
    A2p = np.ascontiguousarray(
        (piT2[..., None] == np.arange(128, dtype=f32)).astype(bfloat16)
    )

    # vocab: cast to bf16 and transpose to [p, r, f] so device DMAs are
    # contiguous multi-KB runs per partition (pure data movement + rounding)
    vocabT = np.ascontiguousarray(
        np.asarray(vocab, dtype=f32).astype(bfloat16)
        .reshape(R, 128, FD).transpose(1, 0, 2)
    )

    def wchunks(w):
        # [H] -> [128, H/128]: column c holds weights for h in [128c,128c+128)
        return np.ascontiguousarray(
            np.asarray(w, dtype=f32).reshape(-1).reshape(H // 128, 128).T
        )

    return {
        "ctxT": np.ascontiguousarray(np.asarray(ctx, dtype=f32).T),
        "hidT": np.ascontiguousarray(np.asarray(hid, dtype=f32).T),
        "trgT": np.ascontiguousarray(np.asarray(trg, dtype=f32).T),
        "vocabT": vocabT,
        "Ap": Ap,
        "A2p": A2p,
        "attnT": tr(attn),
        "fiT": tr(fi),
        "attnT2": tr2(attn),
        "fiT2": np.ascontiguousarray(fiT2),
        "whT": wchunks(w_h),
        "wsT": wchunks(w_s),
        "wxT": wchunks(w_x_w),
        "wxb": rep(w_x_b, 1),
        "iotaF": rep(np.arange(FD, dtype=f32), FD).astype(bfloat16),
        "iotaF2": rep(np.arange(2 * FD, dtype=f32), 2 * FD).astype(np.float16),
        "ident": np.eye(128, dtype=np.float32).astype(bfloat16),
    }


def make_in_maps(context_vecs, hidden, trg_embs, vocab_dists, attn_dists,
                 src_ids, w_h, w_s, w_x_w, w_x_b):
    """Build the 8 per-core input dicts from full inputs."""
    context_vecs = np.asarray(context_vecs)
    hidden = np.asarray(hidden)
    trg_embs = np.asarray(trg_embs)
    vocab_dists = np.asarray(vocab_dists)
    attn_dists = np.asarray(attn_dists)
    src_ids = np.asarray(src_ids)

    in_maps = []
    for i in range(N_CORES):
        bs = slice(i * BPC, (i + 1) * BPC)
        in_maps.append(make_core_inputs(
            context_vecs[bs].reshape(R_FULL, H),
            hidden[bs].reshape(R_FULL, H),
            trg_embs[bs].reshape(R_FULL, H),
            vocab_dists[bs].reshape(R_FULL, V),
            attn_dists[bs].reshape(R_FULL, S),
            src_ids[bs].reshape(R_FULL, S),
            w_h, w_s, w_x_w, w_x_b,
        ))
    return in_maps


def kernel(context_vecs, hidden, trg_embs, vocab_dists, attn_dists,
           src_ids, pad_id, w_h, w_s, w_x_w, w_x_b):
    """Full-input entry point. Shards over 8 NeuronCores, returns [B,T,V] f32."""
    from concourse.bass_utils import run_bass_kernel_spmd

    nc = build_program()
    in_maps = make_in_maps(context_vecs, hidden, trg_embs, vocab_dists,
                           attn_dists, src_ids, w_h, w_s, w_x_w, w_x_b)
    res = run_bass_kernel_spmd(nc, in_maps, list(range(N_CORES)))
    outs = []
    for i in range(N_CORES):
        # [128, R, FD] bf16 -> [R, 128*FD] f32
        o = np.asarray(res.results[i]["outT"]).astype(np.float32)
        outs.append(o.transpose(1, 0, 2).reshape(BPC, T, V))
    return np.concatenate(outs, axis=0)
